# revision 1
# baseline (speedup 1.0000x reference)
"""Trainium2 Bass kernel for ChebyNet (K=1) forward pass.

ChebConv with K=1 reduces to a plain linear layer on the T0 (identity) term,
so edge_index / edge_weight never enter the math. The network is:

    h1 = x @ W1.T + b1            -> BN (train mode, over nodes) -> ReLU
    h2 = h1 @ W2.T + b2           -> BN -> ReLU
    h3 = relu(h2 @ Wl1.T + bl1)
    out = log_softmax(h3 @ Wl2.T + bl2, axis=1)

Sharding: nodes (N=50000) split across 8 NeuronCores (6250 rows each).
All compute is node-local except BN statistics:
  - BN1 stats come analytically from an AllReduce of the Gram matrix of x
    plus its column sums (mean/var of x@W1.T are a bilinear form of the
    Gram matrix). One [128,129] f32 AllReduce (~66KB).
  - BN2 stats need post-ReLU activations, so each core accumulates
    sum / sum-of-squares of h2 over its rows and AllReduces [128,16] (8KB).
h2 is spilled to scratch DRAM between the stats pass and the normalize pass.

Activations are stored feature-on-partition ([feat, rows]); BN normalize +
ReLU is one scalar-engine activation with per-partition scale/bias. Matmuls
use float32r (rounded fp32) operands for full-rate PE throughput.
"""

import os
import sys

sys.path.insert(0, "/opt/trn_rl_repo")

import numpy as np

NCORES = 8
N_TOTAL = 50000
R = N_TOTAL // NCORES  # 6250 rows per core
DIN = 128
H = 1024
HM = 256
C = 10
BN_EPS = 1e-5
CH = 512  # row-chunk (matmul moving dim)

NRT = (R + 127) // 128  # 49 row tiles
RT_LIST = [(i * 128, min(128, R - i * 128)) for i in range(NRT)]
CH_LIST = [(i * CH, min(CH, R - i * CH)) for i in range((R + CH - 1) // CH)]
if os.environ.get("CH_LIMIT"):
    CH_LIST = CH_LIST[: int(os.environ["CH_LIMIT"])]

_CACHE = {}


def _build(stage="full"):
    import concourse.bass as bass  # noqa: F401
    import concourse.tile as tile
    import concourse.mybir as mybir
    from concourse import bacc
    from concourse.masks import make_identity

    fp32 = mybir.dt.float32
    f32r = mybir.dt.float32r
    AF = mybir.ActivationFunctionType
    ALU = mybir.AluOpType
    X = mybir.AxisListType.X

    nc = bacc.Bacc(num_devices=NCORES, debug=False)

    x_d = nc.dram_tensor("x", [R, DIN], fp32, kind="ExternalInput")
    xt_d = nc.dram_tensor("xT", [DIN, R], fp32, kind="ExternalInput")
    w1_d = nc.dram_tensor("W1", [H, DIN], fp32, kind="ExternalInput")
    w2_d = nc.dram_tensor("W2", [H, H], fp32, kind="ExternalInput")
    wl1_d = nc.dram_tensor("Wl1", [HM, H], fp32, kind="ExternalInput")
    wl2_d = nc.dram_tensor("Wl2", [C, HM], fp32, kind="ExternalInput")
    # rows: 0=b1 1=g1 2=be1 3=b2 4=g2 5=be2 6=bl1(padded) 7=bl2(padded)
    vecs_d = nc.dram_tensor("vecs", [8, H], fp32, kind="ExternalInput")
    out_d = nc.dram_tensor("out", [R, C], fp32, kind="ExternalOutput")

    NCH = len(CH_LIST)
    RG = [list(range(NCORES))]
    NFULL = R // 128  # full 128-row tiles

    with tile.TileContext(nc) as tc:
        with (
            tc.tile_pool(name="persist", bufs=1) as persist,
            tc.tile_pool(name="work", bufs=2) as work,
            tc.tile_pool(name="dram", bufs=1, space="DRAM") as dram,
        ):
            # ---------------- constants -----------------
            identity = persist.tile([128, 128], fp32, tag="identity", name="identity")
            make_identity(nc, identity[:])
            ones_col = persist.tile([128, 1], fp32, tag="ones", name="ones_col")
            nc.vector.memset(ones_col[:], 1.0)
            ones_row = persist.tile([1, CH], f32r, tag="onesr", name="ones_row")
            ones_row_f = persist.tile([1, CH], fp32, tag="onesrf", name="ones_row_f")
            nc.vector.memset(ones_row_f[:], 1.0)
            nc.scalar.copy(ones_row[:], ones_row_f[:])

            vraw = persist.tile([8, H], fp32, tag="vraw", name="vraw")
            nc.sync.dma_start(out=vraw[:], in_=vecs_d[:])

            w1T = persist.tile([128, H], fp32, tag="w1T", name="w1T")
            w1T_r = persist.tile([128, H], f32r, tag="w1T_r", name="w1T_r")
            w2T = [
                persist.tile([128, H], f32r, tag=f"w2T{k}", name=f"w2T{k}")
                for k in range(8)
            ]
            wl1T = [
                persist.tile([128, HM], f32r, tag=f"wl1T{k}", name=f"wl1T{k}")
                for k in range(8)
            ]
            wl2T = [
                persist.tile([128, C], f32r, tag=f"wl2T{k}", name=f"wl2T{k}")
                for k in range(2)
            ]
            bl2r = persist.tile([1, C], f32r, tag="bl2r", name="bl2r")
            bl2tmp = persist.tile([1, C], fp32, tag="bl2tmp", name="bl2tmp")
            nc.sync.dma_start(out=bl2tmp[:], in_=vecs_d[7:8, 0:C])
            nc.scalar.copy(bl2r[:], bl2tmp[:])
            vcols = [
                persist.tile([128, 8], fp32, tag=f"vcols{k}", name=f"vcols{k}")
                for k in range(8)
            ]
            xT = persist.tile([128, R], f32r, tag="xT", name="xT")
            gram_sb = persist.tile(
                [128, DIN + 1], fp32, tag="gram_sb", name="gram_sb"
            )

            # ============ startup: big loads, transposes, Gram ============
            with tc.tile_pool(name="bigload", bufs=1) as bigload, \
                 tc.tile_pool(name="ptr", bufs=3, space="PSUM") as ptr, \
                 tc.tile_pool(name="pacc", bufs=1, space="PSUM") as pacc:
                # vector params -> per-partition columns
                for k in range(8):
                    vp = ptr.tile([128, 8], fp32, tag="ptr", name=f"vps{k}")
                    nc.tensor.transpose(
                        vp[:], vraw[:, k * 128 : (k + 1) * 128], identity[:8, :8]
                    )
                    nc.scalar.copy(vcols[k][:], vp[:])

                # ---- bulk loads ----
                # x first (it gates the Gram -> AllReduce critical path),
                # split across the three DMA-issue engines.
                xall = bigload.tile([128, NRT * DIN], fp32, tag="xall", name="xall")
                x_eng = [nc.sync, nc.scalar, nc.gpsimd]
                npieces = 6
                step = (NFULL + npieces - 1) // npieces
                for bi in range(npieces):
                    ta, tb = bi * step, min((bi + 1) * step, NFULL)
                    if ta >= tb:
                        continue
                    x_eng[bi % 3].dma_start(
                        out=xall[:, ta * DIN : tb * DIN],
                        in_=x_d[ta * 128 : tb * 128].rearrange(
                            "(t p) d -> p t d", p=128
                        ),
                    )
                rtail = R - NFULL * 128
                if rtail:
                    nc.sync.dma_start(
                        out=xall[:rtail, NFULL * DIN :],
                        in_=x_d[NFULL * 128 :, :],
                    )

                w1load = bigload.tile([128, H], fp32, tag="w1load", name="w1load")
                nc.sync.dma_start(
                    out=w1load[:],
                    in_=w1_d[:].rearrange("(t p) d -> p t d", p=128),
                )
                w2load = bigload.tile([128, 8 * H], fp32, tag="w2load", name="w2load")
                nc.sync.dma_start(
                    out=w2load[:, : 4 * H],
                    in_=w2_d[: 4 * 128].rearrange("(t p) d -> p t d", p=128),
                )
                nc.scalar.dma_start(
                    out=w2load[:, 4 * H :],
                    in_=w2_d[4 * 128 :].rearrange("(t p) d -> p t d", p=128),
                )
                wl1load = bigload.tile(
                    [128, 2 * H], fp32, tag="wl1load", name="wl1load"
                )
                nc.gpsimd.dma_start(
                    out=wl1load[:],
                    in_=wl1_d[:].rearrange("(t p) d -> p t d", p=128),
                )
                wl2load = bigload.tile([C, HM], fp32, tag="wl2load", name="wl2load")
                nc.sync.dma_start(out=wl2load[:], in_=wl2_d[:])

                # ---- xT: host-transposed, rounded to f32r on device ----
                xTf = bigload.tile([128, R], fp32, tag="xTf", name="xTf")
                half = (R // 2) // 128 * 128
                nc.sync.dma_start(out=xTf[:, :half], in_=xt_d[:, :half])
                nc.scalar.dma_start(out=xTf[:, half:], in_=xt_d[:, half:])
                nc.scalar.copy(xT[:, : R // 2], xTf[:, : R // 2])
                nc.scalar.copy(xT[:, R // 2 :], xTf[:, R // 2 :])

                gram_ps = pacc.tile([128, DIN], fp32, tag="gram", name="gram_ps")
                cs_ps = pacc.tile([128, 1], fp32, tag="cs", name="cs_ps")
                for i, (r0, rr) in enumerate(RT_LIST):
                    sl = slice(i * DIN, (i + 1) * DIN)
                    nc.tensor.matmul(
                        gram_ps[:],
                        lhsT=xall[:rr, sl],
                        rhs=xall[:rr, sl],
                        start=(i == 0),
                        stop=(i == NRT - 1),
                    )
                for i, (r0, rr) in enumerate(RT_LIST):
                    sl = slice(i * DIN, (i + 1) * DIN)
                    nc.tensor.matmul(
                        cs_ps[:],
                        lhsT=xall[:rr, sl],
                        rhs=ones_col[:rr, :],
                        start=(i == 0),
                        stop=(i == NRT - 1),
                    )

                stats1_sb = work.tile(
                    [128, DIN + 1], fp32, tag="st1", name="stats1_sb"
                )
                nc.vector.tensor_copy(stats1_sb[:, 0:DIN], gram_ps[:])
                nc.vector.tensor_copy(stats1_sb[:, DIN : DIN + 1], cs_ps[:])

                cc1_in = dram.tile([128, DIN + 1], fp32, name="cc1_in")
                cc1_out = dram.tile([128, DIN + 1], fp32, name="cc1_out")
                nc.sync.dma_start(out=cc1_in[:], in_=stats1_sb[:])
                nc.gpsimd.collective_compute(
                    "AllReduce",
                    ALU.add,
                    replica_groups=RG,
                    ins=[cc1_in[:].opt()],
                    outs=[cc1_out[:].opt()],
                )
                nc.sync.dma_start(out=gram_sb[:], in_=cc1_out[:])

                # ---- weight transposes (overlap the AllReduce wait) ----
                for m in range(8):
                    tp = ptr.tile([128, 128], fp32, tag="ptr", name=f"w1ps{m}")
                    nc.tensor.transpose(
                        tp[:], w1load[:, m * DIN : (m + 1) * DIN], identity[:]
                    )
                    nc.scalar.copy(w1T[:, m * 128 : (m + 1) * 128], tp[:])
                nc.scalar.copy(w1T_r[:], w1T[:])

                for m in range(8):
                    for k in range(8):
                        tp = ptr.tile([128, 128], fp32, tag="ptr", name=f"w2ps{m}_{k}")
                        nc.tensor.transpose(
                            tp[:],
                            w2load[:, m * H + k * 128 : m * H + (k + 1) * 128],
                            identity[:],
                        )
                        nc.vector.tensor_copy(
                            w2T[k][:, m * 128 : (m + 1) * 128], tp[:]
                        )

                for m in range(2):
                    for k in range(8):
                        tp = ptr.tile(
                            [128, 128], fp32, tag="ptr", name=f"wl1ps{m}_{k}"
                        )
                        nc.tensor.transpose(
                            tp[:],
                            wl1load[:, m * H + k * 128 : m * H + (k + 1) * 128],
                            identity[:],
                        )
                        nc.scalar.copy(wl1T[k][:, m * 128 : (m + 1) * 128], tp[:])

                for k in range(2):
                    tp = ptr.tile([128, C], fp32, tag="ptr", name=f"wl2ps{k}")
                    nc.tensor.transpose(
                        tp[:],
                        wl2load[:, k * 128 : (k + 1) * 128],
                        identity[:10, :10],
                    )
                    nc.scalar.copy(wl2T[k][:], tp[:])

            if stage == "s1":
                dummy = work.tile([128, C], fp32, tag="dummy", name="dummy")
                nc.vector.tensor_copy(dummy[:], gram_sb[:, 0:C])
                for r0 in range(0, R, 128):
                    rr = min(128, R - r0)
                    nc.sync.dma_start(out=out_d[r0 : r0 + rr, :], in_=dummy[:rr, :])
            else:
                _build_rest(
                    nc, tc, stage, mybir, fp32, AF, ALU, X,
                    persist, work, dram, identity, ones_col, ones_row, bl2r,
                    vcols, w1T, w1T_r, w2T, wl1T, wl2T, xT, gram_sb, out_d,
                    NCH, RG,
                )

    nc.finalize()
    return nc


def _build_rest(
    nc, tc, stage, mybir, fp32, AF, ALU, X,
    persist, work, dram, identity, ones_col, ones_row, bl2r,
    vcols, w1T, w1T_r, w2T, wl1T, wl2T, xT, gram_sb, out_d, NCH, RG,
):
    f32r = mybir.dt.float32r

    # ------------------- BN1 statistics --------------------
    bn1_scale = persist.tile([128, 8], fp32, tag="bn1s", name="bn1_scale")
    bn1_bias = persist.tile([128, 8], fp32, tag="bn1b", name="bn1_bias")

    with tc.tile_pool(name="pbigC", bufs=2, space="PSUM") as pbigC, \
         tc.tile_pool(name="psmall", bufs=4, space="PSUM") as psmall:
        mean_x = work.tile([128, 1], fp32, tag="meanx", name="mean_x")
        nc.scalar.mul(mean_x[:], gram_sb[:, DIN : DIN + 1], 1.0 / N_TOTAL)
        V_sb = work.tile([128, H], fp32, tag="Vsb", name="V_sb")
        for half in range(2):
            mp = pbigC.tile([128, 512], fp32, tag="pbigC", name=f"mp{half}")
            nc.tensor.matmul(
                mp[:],
                lhsT=gram_sb[:, 0:DIN],
                rhs=w1T[:, half * 512 : (half + 1) * 512],
                start=True,
                stop=True,
            )
            nc.vector.tensor_mul(
                V_sb[:, half * 512 : (half + 1) * 512],
                w1T[:, half * 512 : (half + 1) * 512],
                mp[:],
            )
        for m in range(8):
            sl = slice(m * 128, (m + 1) * 128)
            e2_ps = psmall.tile([128, 1], fp32, tag="psmall", name=f"e2{m}")
            nc.tensor.matmul(
                e2_ps[:], lhsT=V_sb[:, sl], rhs=ones_col[:],
                start=True, stop=True,
            )
            wxm_ps = psmall.tile([128, 1], fp32, tag="psmall", name=f"wxm{m}")
            nc.tensor.matmul(
                wxm_ps[:], lhsT=w1T[:, sl], rhs=mean_x[:],
                start=True, stop=True,
            )
            wxm_sb = work.tile([128, 1], fp32, tag="wxmsb", name=f"wxmsb{m}")
            nc.scalar.copy(wxm_sb[:], wxm_ps[:])
            var_t = work.tile([128, 1], fp32, tag="var", name=f"var{m}")
            nc.vector.tensor_scalar_mul(var_t[:], e2_ps[:], 1.0 / N_TOTAL)
            msq = work.tile([128, 1], fp32, tag="msq", name=f"msq{m}")
            nc.vector.tensor_mul(msq[:], wxm_sb[:], wxm_sb[:])
            nc.vector.tensor_sub(var_t[:], var_t[:], msq[:])
            nc.vector.tensor_scalar_add(var_t[:], var_t[:], BN_EPS)
            sd = work.tile([128, 1], fp32, tag="sd", name=f"sd{m}")
            nc.scalar.sqrt(sd[:], var_t[:])
            rstd = work.tile([128, 1], fp32, tag="rstd", name=f"rstd{m}")
            nc.vector.reciprocal(rstd[:], sd[:])
            nc.vector.tensor_mul(
                bn1_scale[:, m : m + 1], rstd[:], vcols[m][:, 1:2]
            )
            t2 = work.tile([128, 1], fp32, tag="t2", name=f"t2{m}")
            nc.vector.tensor_mul(t2[:], wxm_sb[:], bn1_scale[:, m : m + 1])
            nc.vector.tensor_sub(
                bn1_bias[:, m : m + 1], vcols[m][:, 2:3], t2[:]
            )

    if stage == "s1b":
        dummy = work.tile([128, C], fp32, tag="dummy", name="dummy")
        nc.vector.tensor_copy(dummy[:, 0:8], bn1_scale[:])
        nc.vector.tensor_copy(dummy[:, 8:10], bn1_bias[:, 0:2])
        for r0 in range(0, R, 128):
            rr = min(128, R - r0)
            nc.sync.dma_start(out=out_d[r0 : r0 + rr, :], in_=dummy[:rr, :])
        return

    # ------------- main pass: L1 -> BN1+ReLU -> L2 ------------
    sum_parts = [
        persist.tile([128, NCH], fp32, tag=f"sump{m}", name=f"sump{m}")
        for m in range(8)
    ]
    sumsq_parts = [
        persist.tile([128, NCH], fp32, tag=f"sumq{m}", name=f"sumq{m}")
        for m in range(8)
    ]
    h2_dram = dram.tile([8, 128, R], fp32, name="h2_dram")

    with (
        tc.tile_pool(name="acts", bufs=1) as acts,
        tc.tile_pool(name="h2stage", bufs=4) as h2stage,
        tc.tile_pool(name="h2load", bufs=1) as h2load,
        tc.tile_pool(name="sqs", bufs=3) as sqs,
        tc.tile_pool(name="h3pool", bufs=1) as h3pool,
        tc.tile_pool(name="lgpool", bufs=2) as lgpool,
    ):
        with tc.tile_pool(name="ph1", bufs=2, space="PSUM") as ph1, \
             tc.tile_pool(name="ph2", bufs=2, space="PSUM") as ph2:
            for g in range(0, NCH, 2):
                pair = list(enumerate(CH_LIST))[g : g + 2]
                a1 = {}
                for j, (c0, cc) in pair:
                    a1[j] = [
                        acts.tile(
                            [128, CH], f32r, tag=f"act{j & 1}_{k}",
                            name=f"a1_{j}_{k}",
                        )
                        for k in range(8)
                    ]
                # L1: each w1T slice loaded once per pair
                for m in range(8):
                    sl = slice(m * 128, (m + 1) * 128)
                    for j, (c0, cc) in pair:
                        h1_ps = ph1.tile(
                            [128, CH], fp32, tag="ph1", name=f"h1ps{j}_{m}"
                        )
                        nc.tensor.matmul(
                            h1_ps[:, :cc],
                            lhsT=w1T_r[:, sl],
                            rhs=xT[:, c0 : c0 + cc],
                            start=True,
                            stop=True,
                        )
                        nc.scalar.activation(
                            a1[j][m][:, :cc],
                            h1_ps[:, :cc],
                            AF.Relu,
                            bias=bn1_bias[:, m : m + 1],
                            scale=bn1_scale[:, m : m + 1],
                        )
                # L2: each w2T slice loaded once per (k, pair)
                for m in range(8):
                    sl = slice(m * 128, (m + 1) * 128)
                    h2_ps = {}
                    for j, (c0, cc) in pair:
                        h2_ps[j] = ph2.tile(
                            [128, CH], fp32, tag=f"ph2{j & 1}",
                            name=f"h2ps{j}_{m}",
                        )
                    for k in range(8):
                        for j, (c0, cc) in pair:
                            nc.tensor.matmul(
                                h2_ps[j][:, :cc],
                                lhsT=w2T[k][:, sl],
                                rhs=a1[j][k][:, :cc],
                                start=(k == 0),
                                stop=(k == 7),
                            )
                    for j, (c0, cc) in pair:
                        h2s = h2stage.tile(
                            [128, CH], fp32, tag="h2s", name=f"h2s{j}_{m}"
                        )
                        nc.scalar.activation(
                            h2s[:, :cc],
                            h2_ps[j][:, :cc],
                            AF.Identity,
                            bias=0.0,
                            scale=1.0,
                            accum_out=sum_parts[m][:, j : j + 1],
                        )
                        sq = sqs.tile(
                            [128, CH], fp32, tag="sq", name=f"sq{j}_{m}"
                        )
                        nc.vector.tensor_mul(
                            sq[:, :cc], h2s[:, :cc], h2s[:, :cc]
                        )
                        nc.vector.reduce_sum(
                            sumsq_parts[m][:, j : j + 1], sq[:, :cc], axis=X
                        )
                        nc.sync.dma_start(
                            out=h2_dram[m, :, c0 : c0 + cc], in_=h2s[:, :cc]
                        )

        # ---------------- BN2 statistics ----------------
        stats2_sb = work.tile([128, 16], fp32, tag="st2", name="stats2_sb")
        for m in range(8):
            nc.vector.reduce_sum(
                stats2_sb[:, m : m + 1], sum_parts[m][:], axis=X
            )
            nc.vector.reduce_sum(
                stats2_sb[:, 8 + m : 9 + m], sumsq_parts[m][:], axis=X
            )

        cc2_in = dram.tile([128, 16], fp32, name="cc2_in")
        cc2_out = dram.tile([128, 16], fp32, name="cc2_out")
        nc.sync.dma_start(out=cc2_in[:], in_=stats2_sb[:])
        nc.gpsimd.collective_compute(
            "AllReduce",
            ALU.add,
            replica_groups=RG,
            ins=[cc2_in[:].opt()],
            outs=[cc2_out[:].opt()],
        )
        stats2g = work.tile([128, 16], fp32, tag="st2g", name="stats2g")
        nc.sync.dma_start(out=stats2g[:], in_=cc2_out[:])

        bn2_scale = persist.tile([128, 8], fp32, tag="bn2s", name="bn2_scale")
        bn2_bias = persist.tile([128, 8], fp32, tag="bn2b", name="bn2_bias")
        for m in range(8):
            mean2 = work.tile([128, 1], fp32, tag="mean2", name=f"mean2_{m}")
            nc.scalar.mul(mean2[:], stats2g[:, m : m + 1], 1.0 / N_TOTAL)
            var_t = work.tile([128, 1], fp32, tag="var2", name=f"var2_{m}")
            nc.scalar.mul(
                var_t[:], stats2g[:, 8 + m : 9 + m], 1.0 / N_TOTAL
            )
            msq = work.tile([128, 1], fp32, tag="msq2", name=f"msq2_{m}")
            nc.vector.tensor_mul(msq[:], mean2[:], mean2[:])
            nc.vector.tensor_sub(var_t[:], var_t[:], msq[:])
            nc.vector.tensor_scalar_add(var_t[:], var_t[:], BN_EPS)
            sd = work.tile([128, 1], fp32, tag="sd2", name=f"sd2_{m}")
            nc.scalar.sqrt(sd[:], var_t[:])
            rstd = work.tile([128, 1], fp32, tag="rstd2", name=f"rstd2_{m}")
            nc.vector.reciprocal(rstd[:], sd[:])
            nc.vector.tensor_mul(
                bn2_scale[:, m : m + 1], rstd[:], vcols[m][:, 4:5]
            )
            t2 = work.tile([128, 1], fp32, tag="t22", name=f"t22_{m}")
            nc.vector.tensor_mul(t2[:], mean2[:], bn2_scale[:, m : m + 1])
            nc.vector.tensor_sub(
                bn2_bias[:, m : m + 1], vcols[m][:, 5:6], t2[:]
            )

        # ------ final pass: BN2+ReLU -> L3 -> L4 -> softmax ------
        NRTT = (R + 127) // 128
        NFULL = R // 128
        rows_all = persist.tile(
            [128, NRTT * C], fp32, tag="rows_all", name="rows_all"
        )
        nc.vector.memset(rows_all[:], 0.0)
        e_all = persist.tile([128, NRTT * C], fp32, tag="e_all", name="e_all")
        res_all = persist.tile(
            [128, NRTT * C], fp32, tag="res_all", name="res_all"
        )
        sums_all = persist.tile([128, NRTT], fp32, tag="sums_all", name="sums_all")
        lse_all = persist.tile([128, NRTT], fp32, tag="lse_all", name="lse_all")
        with tc.tile_pool(name="ph3", bufs=3, space="PSUM") as ph3, \
             tc.tile_pool(name="plog", bufs=2, space="PSUM") as plog, \
             tc.tile_pool(name="ptr2", bufs=3, space="PSUM") as ptr2:
            for g in range(0, NCH, 2):
                pair = list(enumerate(CH_LIST))[g : g + 2]
                h2l = {}
                a2 = {}
                for j, (c0, cc) in pair:
                    h2l[j] = [
                        h2load.tile(
                            [128, CH], fp32, tag=f"h2l{j & 1}_{k}",
                            name=f"h2l{j}_{k}",
                        )
                        for k in range(8)
                    ]
                    a2[j] = [
                        acts.tile(
                            [128, CH], f32r, tag=f"act{j & 1}_{k}",
                            name=f"a2_{j}_{k}",
                        )
                        for k in range(8)
                    ]
                    for k in range(8):
                        nc.sync.dma_start(
                            out=h2l[j][k][:, :cc],
                            in_=h2_dram[k, :, c0 : c0 + cc],
                        )
                        if k < 4:
                            nc.scalar.activation(
                                a2[j][k][:, :cc],
                                h2l[j][k][:, :cc],
                                AF.Relu,
                                bias=bn2_bias[:, k : k + 1],
                                scale=bn2_scale[:, k : k + 1],
                            )
                        else:
                            tmp = sqs.tile(
                                [128, CH], fp32, tag="sq", name=f"af{j}_{k}"
                            )
                            nc.vector.tensor_scalar(
                                out=tmp[:, :cc],
                                in0=h2l[j][k][:, :cc],
                                scalar1=bn2_scale[:, k : k + 1],
                                scalar2=bn2_bias[:, k : k + 1],
                                op0=ALU.mult,
                                op1=ALU.add,
                            )
                            nc.vector.tensor_scalar_max(
                                a2[j][k][:, :cc], tmp[:, :cc], 0.0
                            )
                h3 = {}
                for j, (c0, cc) in pair:
                    h3[j] = [
                        h3pool.tile(
                            [128, CH], f32r, tag=f"h3_{j & 1}_{m3}",
                            name=f"h3_{j}_{m3}",
                        )
                        for m3 in range(2)
                    ]
                for m3 in range(2):
                    sl = slice(m3 * 128, (m3 + 1) * 128)
                    h3_ps = {}
                    for j, (c0, cc) in pair:
                        h3_ps[j] = ph3.tile(
                            [128, CH], fp32, tag="ph3", name=f"h3ps{j}_{m3}"
                        )
                    for k in range(8):
                        for j, (c0, cc) in pair:
                            nc.tensor.matmul(
                                h3_ps[j][:, :cc],
                                lhsT=wl1T[k][:, sl],
                                rhs=a2[j][k][:, :cc],
                                start=(k == 0),
                                stop=(k == 7),
                            )
                    for j, (c0, cc) in pair:
                        nc.vector.tensor_scalar(
                            out=h3[j][m3][:, :cc],
                            in0=h3_ps[j][:, :cc],
                            scalar1=vcols[m3][:, 6:7],
                            scalar2=0.0,
                            op0=ALU.add,
                            op1=ALU.max,
                        )
                for j, (c0, cc) in pair:
                    lg_ps = plog.tile([C, CH], fp32, tag="plog", name=f"lg{j}")
                    nc.tensor.matmul(
                        lg_ps[:, :cc],
                        lhsT=bl2r[:],
                        rhs=ones_row[:, :cc],
                        start=True,
                        stop=False,
                    )
                    for k in range(2):
                        nc.tensor.matmul(
                            lg_ps[:, :cc],
                            lhsT=wl2T[k][:],
                            rhs=h3[j][k][:, :cc],
                            start=False,
                            stop=(k == 1),
                        )
                    lg_sb = lgpool.tile([C, CH], fp32, tag="lg", name=f"lgs{j}")
                    nc.vector.tensor_copy(lg_sb[:, :cc], lg_ps[:, :cc])
                    # transpose logits to row-major and collect into rows_all
                    nt = (cc + 127) // 128
                    for t in range(nt):
                        rt0 = t * 128
                        rt = min(128, cc - rt0)
                        tg = (c0 + rt0) // 128
                        tp_ps = ptr2.tile(
                            [128, C], fp32, tag="ptr2", name=f"sm{j}_{t}"
                        )
                        nc.tensor.transpose(
                            tp_ps[:rt, :],
                            lg_sb[:, rt0 : rt0 + rt],
                            identity[:C, :C],
                        )
                        nc.vector.tensor_copy(
                            rows_all[:rt, tg * C : (tg + 1) * C], tp_ps[:rt, :]
                        )

            # ---- batched log_softmax over all row tiles ----
            # logits are O(10), so exp() without max-subtraction is safe in f32
            nc.scalar.activation(e_all[:], rows_all[:], AF.Exp)
            nc.vector.reduce_sum(
                sums_all[:],
                e_all[:].rearrange("p (t c) -> p t c", c=C),
                axis=X,
            )
            nc.scalar.activation(lse_all[:], sums_all[:], AF.Ln)
            nc.vector.tensor_sub(
                res_all[:].rearrange("p (t c) -> p t c", c=C),
                rows_all[:].rearrange("p (t c) -> p t c", c=C),
                lse_all[:].to_broadcast([128, NRTT, C]),
            )
            nc.sync.dma_start(
                out=out_d[: NFULL * 128].rearrange("(t p) c -> p t c", p=128),
                in_=res_all[:, : NFULL * C],
            )
            rtail = R - NFULL * 128
            if rtail:
                nc.sync.dma_start(
                    out=out_d[NFULL * 128 :],
                    in_=res_all[:rtail, NFULL * C :],
                )


def _get_nc():
    if "nc" not in _CACHE:
        _CACHE["nc"] = _build(os.environ.get("KERNEL_STAGE", "full"))
    return _CACHE["nc"]


def kernel(**inputs):
    from concourse.bass_utils import run_bass_kernel_spmd

    f32 = np.float32
    x = np.ascontiguousarray(np.asarray(inputs["x"]), dtype=f32)
    W1 = np.ascontiguousarray(np.asarray(inputs["W1"]), dtype=f32)
    W2 = np.ascontiguousarray(np.asarray(inputs["W2"]), dtype=f32)
    Wl1 = np.ascontiguousarray(np.asarray(inputs["Wl1"]), dtype=f32)
    Wl2 = np.ascontiguousarray(np.asarray(inputs["Wl2"]), dtype=f32)
    vecs = np.zeros((8, H), f32)
    vecs[0, :] = np.asarray(inputs["b1"], dtype=f32)
    vecs[1, :] = np.asarray(inputs["g1"], dtype=f32)
    vecs[2, :] = np.asarray(inputs["be1"], dtype=f32)
    vecs[3, :] = np.asarray(inputs["b2"], dtype=f32)
    vecs[4, :] = np.asarray(inputs["g2"], dtype=f32)
    vecs[5, :] = np.asarray(inputs["be2"], dtype=f32)
    vecs[6, :HM] = np.asarray(inputs["bl1"], dtype=f32)
    vecs[7, :C] = np.asarray(inputs["bl2"], dtype=f32)

    nc = _get_nc()
    in_maps = [
        {
            "x": x[i * R : (i + 1) * R],
            "xT": np.ascontiguousarray(x[i * R : (i + 1) * R].T),
            "W1": W1,
            "W2": W2,
            "Wl1": Wl1,
            "Wl2": Wl2,
            "vecs": vecs,
        }
        for i in range(NCORES)
    ]
    res = run_bass_kernel_spmd(nc, in_maps, core_ids=list(range(NCORES)))
    return np.concatenate([r["out"] for r in res.results], axis=0).astype(f32)



# revision 9
# speedup vs baseline: 1.3918x; 1.3918x over previous
"""Trainium2 Bass kernel for ChebyNet (K=1) forward pass.

ChebConv with K=1 reduces to a plain linear layer on the T0 (identity) term,
so edge_index / edge_weight never enter the math. The network is:

    h1 = x @ W1.T (+b1)           -> BN (train mode, over nodes) -> ReLU
    h2 = a1 @ W2.T (+b2)          -> BN -> ReLU
    h3 = relu(h2 @ Wl1.T + bl1)
    out = log_softmax(h3 @ Wl2.T + bl2, axis=1)

(b1/b2 cancel exactly inside train-mode BN and are dropped.)

Sharding: nodes (N=50000) split across 8 NeuronCores (6250 rows each).
Everything is computed feature-on-partition ([feat, rows]).

Design (vs the 516-593us v0 baseline):
 - All transposes / dtype packing on host: xT, x row-tiles (with a ones
   column for the column-sum), W1T, W2T/Wl1T/Wl2T, per-partition BN
   parameter columns. No on-device transposes.
 - bf16 operands everywhere on the matmul path (fp8 DoubleRow was measured
   at rel_err 2.6-3.4e-2 for the K=1024 layers - over the 2e-2 gate - so
   bf16 it is; same PE cols/cycle as f32r but half the SBUF/DMA).
 - BN1 stats analytically from the Gram matrix of x, projected locally to
   diag(W1 G W1^T) BEFORE the AllReduce -> payload [128,16] (8KB).
   Collectives serialize on the CC stream with ~30us latency each, so no
   extra warm-up AR (it would delay AR1).
 - During the AR1 wait, L1 (pre-BN) is computed for all chunks into SBUF
   (bf16). The main pass applies BN1+ReLU, runs L2, and overwrites h1 with
   h2 *in place* in SBUF - h2 never spills to DRAM.
 - BN2 stats: sum(h2) via vector tensor_scalar accum_out during the
   PSUM->SBUF copy; sumsq(h2) split scalar(Square+accum)/vector(STT+accum).
 - log_softmax stays feature-major: exp/colsum/ln + a K=1 matmul that adds
   -ln(sum) back into the logits PSUM; output is [10, R]; host transposes
   to [R, 10] (layout-only, like the xT input).
"""

import os
import sys

sys.path.insert(0, "/opt/trn_rl_repo")

import numpy as np
import ml_dtypes

NCORES = 8
N_TOTAL = 50000
R = N_TOTAL // NCORES  # 6250 rows per core
DIN = 128
H = 1024
HM = 256
C = 10
BN_EPS = 1e-5

CH = 1024  # main-pass row chunk
FCH = 512  # final-pass row chunk
CH_LIST = [(i * CH, min(CH, R - i * CH)) for i in range((R + CH - 1) // CH)]
FCH_LIST = [(i * FCH, min(FCH, R - i * FCH)) for i in range((R + FCH - 1) // FCH)]
if os.environ.get("CH_LIMIT"):
    CH_LIST = CH_LIST[: int(os.environ["CH_LIMIT"])]
NCH = len(CH_LIST)

NRT = (R + 127) // 128  # 49 row tiles for the Gram matrix
D1 = DIN + 1  # x tile width incl the ones column

L2_FP8 = os.environ.get("L2_FP8", "0") == "1"
L3_FP8 = os.environ.get("L3_FP8", "0") == "1"

_CACHE = {}


def _halves(cc, step=512):
    out = []
    off = 0
    while off < cc:
        out.append((off, min(step, cc - off)))
        off += step
    return out


def _build(stage="full"):
    import concourse.bass as bass  # noqa: F401
    import concourse.tile as tile
    import concourse.mybir as mybir
    from concourse import bacc

    fp32 = mybir.dt.float32
    f32r = mybir.dt.float32r
    bf16 = mybir.dt.bfloat16
    fp8 = mybir.dt.float8e4
    AF = mybir.ActivationFunctionType
    ALU = mybir.AluOpType
    X = mybir.AxisListType.X
    DR = mybir.MatmulPerfMode.DoubleRow

    l2dt = fp8 if L2_FP8 else bf16
    l3dt = fp8 if L3_FP8 else bf16

    nc = bacc.Bacc(num_devices=NCORES, debug=False)

    x2_d = nc.dram_tensor("x2", [128, NRT * D1], bf16, kind="ExternalInput")
    xt_d = nc.dram_tensor("xT", [128, R], bf16, kind="ExternalInput")
    w1f_d = nc.dram_tensor("w1f", [128, H], f32r, kind="ExternalInput")
    w1b_d = nc.dram_tensor("w1b", [128, H], bf16, kind="ExternalInput")
    w2p_d = nc.dram_tensor("w2p", [128, 8 * H], l2dt, kind="ExternalInput")
    wl1p_d = nc.dram_tensor("wl1p", [128, 8 * HM], l3dt, kind="ExternalInput")
    wl2t_d = nc.dram_tensor("wl2t", [128, 2 * C], bf16, kind="ExternalInput")
    vc_d = nc.dram_tensor("vc", [128, 64], fp32, kind="ExternalInput")
    bl2c_d = nc.dram_tensor("bl2c", [16, 1], fp32, kind="ExternalInput")
    ones_d = nc.dram_tensor("ones", [128, 1], f32r, kind="ExternalInput")
    mones_d = nc.dram_tensor("mones", [1, 16], f32r, kind="ExternalInput")
    out_d = nc.dram_tensor("out", [C, R], fp32, kind="ExternalOutput")

    RG = [list(range(NCORES))]

    with tile.TileContext(nc) as tc:
        with (
            tc.tile_pool(name="persist", bufs=1) as persist,
            tc.tile_pool(name="dram", bufs=1, space="DRAM") as dram,
        ):
            # ---------------- persistent tiles -----------------
            hbuf = [
                persist.tile([128, R], bf16, tag=f"hb{m}", name=f"hbuf{m}")
                for m in range(8)
            ]
            w1bs = persist.tile([128, H], bf16, tag="w1bs", name="w1bs")
            w2ps = persist.tile([128, 8 * H], l2dt, tag="w2ps", name="w2ps")
            wl1ps = persist.tile([128, 8 * HM], l3dt, tag="wl1ps", name="wl1ps")
            wl2ts = persist.tile([128, 2 * C], bf16, tag="wl2ts", name="wl2ts")
            vcs = persist.tile([128, 64], fp32, tag="vcs", name="vcs")
            bl2cs = persist.tile([16, 1], fp32, tag="bl2cs", name="bl2cs")
            ones_r = persist.tile([128, 1], f32r, tag="ones_r", name="ones_r")
            mones10 = persist.tile([1, 16], f32r, tag="mones", name="mones10")
            bn1_s = persist.tile([128, 8], fp32, tag="bn1s", name="bn1_s")
            bn1_b = persist.tile([128, 8], fp32, tag="bn1b", name="bn1_b")
            bn2_s = persist.tile([128, 8], fp32, tag="bn2s", name="bn2_s")
            bn2_b = persist.tile([128, 8], fp32, tag="bn2b", name="bn2_b")
            eps_c = persist.tile([128, 1], fp32, tag="epsc", name="eps_c")
            suma = persist.tile([128, 8 * NCH], fp32, tag="suma", name="suma")
            sumq = persist.tile([128, 8 * NCH], fp32, tag="sumq", name="sumq")
            st1g = persist.tile([128, 16], fp32, tag="st1g", name="st1g")
            st2g = persist.tile([128, 16], fp32, tag="st2g", name="st2g")

            cc1_in = dram.tile([128, 16], fp32, name="cc1_in")
            cc1_out = dram.tile([128, 16], fp32, name="cc1_out")
            cc2_in = dram.tile([128, 16], fp32, name="cc2_in")
            cc2_out = dram.tile([128, 16], fp32, name="cc2_out")

            nc.vector.memset(eps_c[:], BN_EPS)
            nc.scalar.dma_start(out=ones_r[:], in_=ones_d[:])
            nc.scalar.dma_start(out=mones10[:], in_=mones_d[:])

            # per-partition views of the BN parameter columns
            vcv = vcs[:].rearrange("p (m j) -> p j m", j=8)  # [128, j, m]

            # ============ startup + prefill (xT lives only here) ============
            with tc.tile_pool(name="xtp", bufs=1) as xtp:
                xTs = xtp.tile([128, R], bf16, tag="xTs", name="xTs")

                with (
                    tc.tile_pool(name="startsb", bufs=1) as startsb,
                    tc.tile_pool(name="startps", bufs=1, space="PSUM") as startps,
                ):
                    x2s = startsb.tile(
                        [128, NRT * D1], bf16, tag="x2s", name="x2s"
                    )
                    w1fr = startsb.tile([128, H], f32r, tag="w1fr", name="w1fr")
                    v_r = startsb.tile([128, H], bf16, tag="v_r", name="v_r")
                    mean_r = startsb.tile([128, 1], bf16, tag="mean_r", name="mean_r")
                    ones_b = startsb.tile([128, 1], bf16, tag="ones_b", name="ones_b")
                    st1sb = startsb.tile([128, 16], fp32, tag="st1sb", name="st1sb")
                    nc.vector.memset(ones_b[:], 1.0)

                    # big loads on the sync DMA queue; small ones on scalar
                    nc.sync.dma_start(out=x2s[:], in_=x2_d[:])
                    nc.sync.dma_start(out=xTs[:], in_=xt_d[:])
                    nc.sync.dma_start(out=w2ps[:], in_=w2p_d[:])
                    nc.sync.dma_start(out=wl1ps[:], in_=wl1p_d[:])
                    nc.scalar.dma_start(out=w1fr[:], in_=w1f_d[:])
                    nc.scalar.dma_start(out=w1bs[:], in_=w1b_d[:])
                    nc.scalar.dma_start(out=vcs[:], in_=vc_d[:])
                    nc.scalar.dma_start(out=wl2ts[:], in_=wl2t_d[:])
                    nc.scalar.dma_start(out=bl2cs[:], in_=bl2c_d[:])

                    # Gram matrix of x (incl ones column -> column sums)
                    gram_ps = startps.tile(
                        [128, D1], fp32, tag="gram", name="gram_ps"
                    )
                    for t in range(NRT):
                        o = t * D1
                        nc.tensor.matmul(
                            gram_ps[:],
                            lhsT=x2s[:, o : o + DIN],
                            rhs=x2s[:, o : o + D1],
                            start=(t == 0),
                            stop=(t == NRT - 1),
                        )
                    gram_r = startsb.tile(
                        [128, D1], f32r, tag="gram_r", name="gram_r"
                    )
                    nc.vector.tensor_copy(gram_r[:], gram_ps[:])
                    nc.scalar.mul(mean_r[:], gram_r[:, DIN : D1], 1.0 / N_TOTAL)

                    # P = G @ W1T ; V = W1T*P ; e2[f]=colsum(V) ; wxm = W1T.T mean
                    st1_ps = startps.tile(
                        [128, 16], fp32, tag="st1ps", name="st1_ps"
                    )
                    for hf in range(2):
                        sl = slice(hf * 512, (hf + 1) * 512)
                        p_ps = startps.tile(
                            [128, 512], fp32, tag=f"pps{hf}", name=f"p_ps{hf}"
                        )
                        nc.tensor.matmul(
                            p_ps[:], lhsT=gram_r[:, 0:DIN], rhs=w1fr[:, sl],
                            start=True, stop=True,
                        )
                        nc.vector.tensor_mul(v_r[:, sl], w1fr[:, sl], p_ps[:])
                    for m in range(8):
                        sl = slice(m * 128, (m + 1) * 128)
                        nc.tensor.matmul(
                            st1_ps[:, m : m + 1], lhsT=v_r[:, sl], rhs=ones_b[:],
                            start=True, stop=True,
                        )
                        nc.tensor.matmul(
                            st1_ps[:, 8 + m : 9 + m], lhsT=w1bs[:, sl],
                            rhs=mean_r[:], start=True, stop=True,
                        )
                    nc.vector.tensor_copy(st1sb[:], st1_ps[:])
                    nc.sync.dma_start(out=cc1_in[:], in_=st1sb[:])
                    nc.gpsimd.collective_compute(
                        "AllReduce", ALU.add, replica_groups=RG,
                        ins=[cc1_in[:].opt()], outs=[cc1_out[:].opt()],
                    )
                    nc.sync.dma_start(out=st1g[:], in_=cc1_out[:])

                # -------- L1 prefill (runs during the AllReduce wait) -------
                with tc.tile_pool(name="ph1", bufs=4, space="PSUM") as ph1:
                    for ci, (c0, cc) in enumerate(CH_LIST):
                        for m in range(8):
                            hp = ph1.tile(
                                [128, CH], fp32, tag="ph1", name=f"h1_{ci}_{m}"
                            )
                            for off, nn in _halves(cc):
                                nc.tensor.matmul(
                                    hp[:, off : off + nn],
                                    lhsT=w1bs[:, m * 128 : (m + 1) * 128],
                                    rhs=xTs[:, c0 + off : c0 + off + nn],
                                    start=True,
                                    stop=True,
                                )
                            if m % 2 == 0:
                                nc.scalar.copy(
                                    hbuf[m][:, c0 : c0 + cc], hp[:, :cc]
                                )
                            else:
                                nc.vector.tensor_copy(
                                    hbuf[m][:, c0 : c0 + cc], hp[:, :cc]
                                )

            # ---------------- BN1 parameters (vectorized) ----------------
            with tc.tile_pool(name="bnw", bufs=1) as bnw:
                var8 = bnw.tile([128, 8], fp32, tag="v8", name="var8")
                msq8 = bnw.tile([128, 8], fp32, tag="m8", name="msq8")
                sd8 = bnw.tile([128, 8], fp32, tag="s8", name="sd8")
                rstd8 = bnw.tile([128, 8], fp32, tag="r8", name="rstd8")
                t8 = bnw.tile([128, 8], fp32, tag="t8", name="t8")
                nc.vector.tensor_scalar_mul(var8[:], st1g[:, 0:8], 1.0 / N_TOTAL)
                nc.vector.tensor_mul(msq8[:], st1g[:, 8:16], st1g[:, 8:16])
                nc.vector.tensor_sub(var8[:], var8[:], msq8[:])
                nc.scalar.activation(sd8[:], var8[:], AF.Sqrt, bias=eps_c[:])
                nc.vector.reciprocal(rstd8[:], sd8[:])
                nc.vector.tensor_mul(bn1_s[:], rstd8[:], vcv[:, 1, :])
                nc.vector.tensor_mul(t8[:], st1g[:, 8:16], bn1_s[:])
                nc.vector.tensor_sub(bn1_b[:], vcv[:, 2, :], t8[:])

            if stage == "s1":
                dummy = persist.tile([16, R], fp32, tag="dummy", name="dummy")
                nc.vector.memset(dummy[:], 0.0)
                nc.vector.tensor_copy(dummy[:10, 0:8], bn1_s[:10, :])
                nc.vector.tensor_copy(dummy[:10, 8:16], bn1_b[:10, :])
                nc.sync.dma_start(out=out_d[:], in_=dummy[:10, :])
                nc.finalize()
                return nc

            # ------------- main pass: BN1+ReLU -> L2 -> stats ------------
            w2v = w2ps[:].rearrange("p (kp two h) -> p kp two h", kp=4, two=2)
            with (
                tc.tile_pool(name="acts", bufs=2) as acts,
                tc.tile_pool(name="sqp", bufs=2) as sqp,
                tc.tile_pool(name="ph2", bufs=2, space="PSUM") as ph2,
            ):
                for ci, (c0, cc) in enumerate(CH_LIST):
                    sl = slice(c0, c0 + cc)
                    a1 = acts.tile(
                        [128, 8 * CH], l2dt, tag=f"act{ci & 1}", name=f"a1_{ci}"
                    )
                    a1v = a1[:].rearrange("p (k c) -> p k c", k=8)
                    for m in range(8):
                        nc.scalar.activation(
                            a1[:, m * CH : m * CH + cc],
                            hbuf[m][:, sl],
                            AF.Relu,
                            bias=bn1_b[:, m : m + 1],
                            scale=bn1_s[:, m : m + 1],
                        )
                    for m2 in range(8):
                        hp = ph2.tile([128, CH], fp32, tag="ph2", name=f"h2_{ci}_{m2}")
                        msl = slice(m2 * 128, (m2 + 1) * 128)
                        if L2_FP8:
                            for off, nn in _halves(cc):
                                for kp in range(4):
                                    nc.tensor.matmul(
                                        hp[:, off : off + nn],
                                        lhsT=w2v[:, kp, :, msl],
                                        rhs=a1v[:, 2 * kp : 2 * kp + 2, off : off + nn],
                                        start=(kp == 0),
                                        stop=(kp == 3),
                                        perf_mode=DR,
                                    )
                        else:
                            for off, nn in _halves(cc):
                                for k in range(8):
                                    nc.tensor.matmul(
                                        hp[:, off : off + nn],
                                        lhsT=w2v[:, k // 2, k % 2, msl],
                                        rhs=a1v[:, k, off : off + nn],
                                        start=(k == 0),
                                        stop=(k == 7),
                                    )
                        # PSUM -> SBUF (bf16, in place over h1) + row-sum
                        nc.vector.tensor_scalar(
                            out=hbuf[m2][:, sl],
                            in0=hp[:, :cc],
                            scalar1=1.0,
                            scalar2=0.0,
                            op0=ALU.mult,
                            op1=ALU.add,
                            accum_out=suma[:, m2 * NCH + ci : m2 * NCH + ci + 1],
                        )
                        sq = sqp.tile(
                            [128, CH], bf16, tag=f"sq{m2 & 1}", name=f"sq_{ci}_{m2}"
                        )
                        if m2 < 3:
                            nc.scalar.activation(
                                sq[:, :cc],
                                hbuf[m2][:, sl],
                                AF.Square,
                                accum_out=sumq[:, m2 * NCH + ci : m2 * NCH + ci + 1],
                            )
                        else:
                            nc.vector.scalar_tensor_tensor(
                                out=sq[:, :cc],
                                in0=hbuf[m2][:, sl],
                                scalar=1.0,
                                in1=hbuf[m2][:, sl],
                                op0=ALU.mult,
                                op1=ALU.mult,
                                accum_out=sumq[:, m2 * NCH + ci : m2 * NCH + ci + 1],
                            )

                # ---------------- BN2 statistics ----------------
                st2sb = sqp.tile([128, 16], fp32, tag="st2sb", name="st2sb")
                sumav = suma[:].rearrange("p (m c) -> p m c", m=8)
                sumqv = sumq[:].rearrange("p (m c) -> p m c", m=8)
                nc.vector.reduce_sum(st2sb[:, 0:8], sumav, axis=X)
                nc.vector.reduce_sum(st2sb[:, 8:16], sumqv, axis=X)
                nc.sync.dma_start(out=cc2_in[:], in_=st2sb[:])
                nc.gpsimd.collective_compute(
                    "AllReduce", ALU.add, replica_groups=RG,
                    ins=[cc2_in[:].opt()], outs=[cc2_out[:].opt()],
                )
                nc.sync.dma_start(out=st2g[:], in_=cc2_out[:])

            # ---------------- BN2 parameters (vectorized) ----------------
            with tc.tile_pool(name="bnw2", bufs=1) as bnw2:
                mean8 = bnw2.tile([128, 8], fp32, tag="n8", name="mean8")
                var8b = bnw2.tile([128, 8], fp32, tag="v8", name="var8b")
                msq8b = bnw2.tile([128, 8], fp32, tag="m8", name="msq8b")
                sd8b = bnw2.tile([128, 8], fp32, tag="s8", name="sd8b")
                rstd8b = bnw2.tile([128, 8], fp32, tag="r8", name="rstd8b")
                t8b = bnw2.tile([128, 8], fp32, tag="t8", name="t8b")
                nc.vector.tensor_scalar_mul(mean8[:], st2g[:, 0:8], 1.0 / N_TOTAL)
                nc.vector.tensor_scalar_mul(var8b[:], st2g[:, 8:16], 1.0 / N_TOTAL)
                nc.vector.tensor_mul(msq8b[:], mean8[:], mean8[:])
                nc.vector.tensor_sub(var8b[:], var8b[:], msq8b[:])
                nc.scalar.activation(sd8b[:], var8b[:], AF.Sqrt, bias=eps_c[:])
                nc.vector.reciprocal(rstd8b[:], sd8b[:])
                nc.vector.tensor_mul(bn2_s[:], rstd8b[:], vcv[:, 4, :])
                nc.vector.tensor_mul(t8b[:], mean8[:], bn2_s[:])
                nc.vector.tensor_sub(bn2_b[:], vcv[:, 5, :], t8b[:])

            # ------ final pass: BN2+ReLU -> L3 -> L4 -> log_softmax ------
            wl1v = wl1ps[:].rearrange("p (kp two h) -> p kp two h", kp=4, two=2)
            with (
                tc.tile_pool(name="acts2", bufs=2) as acts2,
                tc.tile_pool(name="fsb", bufs=2) as fsb,
                tc.tile_pool(name="ph3", bufs=2, space="PSUM") as ph3,
                tc.tile_pool(name="plg", bufs=2, space="PSUM") as plg,
                tc.tile_pool(name="psm", bufs=2, space="PSUM") as psm,
            ):
                for ci, (c0, cc) in enumerate(FCH_LIST):
                    sl = slice(c0, c0 + cc)
                    a2 = acts2.tile(
                        [128, 8 * FCH], l3dt, tag=f"act{ci & 1}", name=f"a2_{ci}"
                    )
                    a2v = a2[:].rearrange("p (k c) -> p k c", k=8)
                    for m in range(8):
                        if m < 6:
                            nc.scalar.activation(
                                a2[:, m * FCH : m * FCH + cc],
                                hbuf[m][:, sl],
                                AF.Relu,
                                bias=bn2_b[:, m : m + 1],
                                scale=bn2_s[:, m : m + 1],
                            )
                        else:
                            tmp = fsb.tile(
                                [128, FCH], bf16, tag=f"tmp{m}", name=f"tmp_{ci}_{m}"
                            )
                            nc.vector.tensor_scalar(
                                out=tmp[:, :cc],
                                in0=hbuf[m][:, sl],
                                scalar1=bn2_s[:, m : m + 1],
                                scalar2=bn2_b[:, m : m + 1],
                                op0=ALU.mult,
                                op1=ALU.add,
                            )
                            nc.vector.tensor_scalar_max(
                                a2[:, m * FCH : m * FCH + cc], tmp[:, :cc], 0.0
                            )
                    h3t = []
                    for m3 in range(2):
                        hp3 = ph3.tile(
                            [128, FCH], fp32, tag="ph3", name=f"h3_{ci}_{m3}"
                        )
                        msl = slice(m3 * 128, (m3 + 1) * 128)
                        if L3_FP8:
                            for kp in range(4):
                                nc.tensor.matmul(
                                    hp3[:, :cc],
                                    lhsT=wl1v[:, kp, :, msl],
                                    rhs=a2v[:, 2 * kp : 2 * kp + 2, :cc],
                                    start=(kp == 0),
                                    stop=(kp == 3),
                                    perf_mode=DR,
                                )
                        else:
                            for k in range(8):
                                nc.tensor.matmul(
                                    hp3[:, :cc],
                                    lhsT=wl1v[:, k // 2, k % 2, msl],
                                    rhs=a2v[:, k, :cc],
                                    start=(k == 0),
                                    stop=(k == 7),
                                )
                        h3 = fsb.tile(
                            [128, FCH], bf16, tag=f"h3_{m3}", name=f"h3t_{ci}_{m3}"
                        )
                        nc.vector.tensor_scalar(
                            out=h3[:, :cc],
                            in0=hp3[:, :cc],
                            scalar1=vcv[:, 6, m3 : m3 + 1],
                            scalar2=0.0,
                            op0=ALU.add,
                            op1=ALU.max,
                        )
                        h3t.append(h3)
                    # L4: logits (pre-bl2) into PSUM
                    lg = plg.tile([C, FCH], fp32, tag="lg", name=f"lg_{ci}")
                    for k in range(2):
                        nc.tensor.matmul(
                            lg[:, :cc],
                            lhsT=wl2ts[:, k * C : (k + 1) * C],
                            rhs=h3t[k][:, :cc],
                            start=(k == 0),
                            stop=False,
                        )
                    # log_softmax: e = exp(lg + bl2); s = colsum(e); lg -= ln s
                    e_sb = fsb.tile([C, FCH], f32r, tag="e_sb", name=f"e_{ci}")
                    lns = fsb.tile([1, FCH], f32r, tag="lns", name=f"lns_{ci}")
                    nc.scalar.activation(
                        e_sb[:, :cc], lg[:, :cc], AF.Exp, bias=bl2cs[:C, :]
                    )
                    sm = psm.tile([1, FCH], fp32, tag="sm", name=f"sm_{ci}")
                    nc.tensor.matmul(
                        sm[:, :cc],
                        lhsT=ones_r[:C, :],
                        rhs=e_sb[:, :cc],
                        start=True,
                        stop=True,
                    )
                    nc.scalar.activation(lns[:, :cc], sm[:, :cc], AF.Ln)
                    nc.tensor.matmul(
                        lg[:, :cc],
                        lhsT=mones10[:, :C],
                        rhs=lns[:, :cc],
                        start=False,
                        stop=True,
                        skip_group_check=True,
                    )
                    osb = fsb.tile([C, FCH], fp32, tag="osb", name=f"osb_{ci}")
                    nc.vector.tensor_scalar(
                        out=osb[:, :cc],
                        in0=lg[:, :cc],
                        scalar1=bl2cs[:C, :],
                        scalar2=0.0,
                        op0=ALU.add,
                        op1=ALU.add,
                    )
                    nc.sync.dma_start(out=out_d[:, sl], in_=osb[:, :cc])

    nc.finalize()
    return nc


def _get_nc():
    if "nc" not in _CACHE:
        _CACHE["nc"] = _build(os.environ.get("KERNEL_STAGE", "full"))
    return _CACHE["nc"]


def make_in_maps(inputs):
    """Host-side layout/dtype prep. Layout-only transforms (transpose, tile,
    pack, cast) - all math happens on device."""
    f32 = np.float32
    bf = ml_dtypes.bfloat16
    f8 = ml_dtypes.float8_e4m3

    x = np.ascontiguousarray(np.asarray(inputs["x"]), dtype=f32)
    W1 = np.asarray(inputs["W1"], dtype=f32)
    W2 = np.asarray(inputs["W2"], dtype=f32)
    Wl1 = np.asarray(inputs["Wl1"], dtype=f32)
    Wl2 = np.asarray(inputs["Wl2"], dtype=f32)

    w1f = np.ascontiguousarray(W1.T)  # [128, H] f32 (f32r on device)
    w1b = w1f.astype(bf)
    l2np = f8 if L2_FP8 else bf
    l3np = f8 if L3_FP8 else bf
    # [128, kp, two, m] packing (DoubleRow-compatible; plain reshape for bf16)
    w2p = np.ascontiguousarray(
        W2.T.reshape(4, 2, 128, H).transpose(2, 0, 1, 3).reshape(128, 8 * H)
    ).astype(l2np)
    wl1p = np.ascontiguousarray(
        Wl1.T.reshape(4, 2, 128, HM).transpose(2, 0, 1, 3).reshape(128, 8 * HM)
    ).astype(l3np)
    wl2t = np.ascontiguousarray(
        Wl2.T.reshape(2, 128, C).transpose(1, 0, 2).reshape(128, 2 * C)
    ).astype(bf)

    vecs = np.zeros((8, H), f32)
    vecs[1, :] = np.asarray(inputs["g1"], dtype=f32)
    vecs[2, :] = np.asarray(inputs["be1"], dtype=f32)
    vecs[4, :] = np.asarray(inputs["g2"], dtype=f32)
    vecs[5, :] = np.asarray(inputs["be2"], dtype=f32)
    vecs[6, :HM] = np.asarray(inputs["bl1"], dtype=f32)
    # vc[p, k, j] = vecs[j, k*128+p]
    vc = np.ascontiguousarray(
        vecs.T.reshape(8, 128, 8).transpose(1, 0, 2).reshape(128, 64)
    )
    bl2c = np.zeros((16, 1), f32)
    bl2c[:C, 0] = np.asarray(inputs["bl2"], dtype=f32)

    in_maps = []
    for i in range(NCORES):
        xs = x[i * R : (i + 1) * R]
        xt = np.ascontiguousarray(xs.T).astype(bf)
        x2 = np.zeros((NRT * 128, D1), f32)
        x2[:R, :DIN] = xs
        x2[:R, DIN] = 1.0
        x2 = np.ascontiguousarray(
            x2.reshape(NRT, 128, D1).transpose(1, 0, 2).reshape(128, NRT * D1)
        ).astype(bf)
        in_maps.append(
            {
                "x2": x2, "xT": xt, "w1f": w1f, "w1b": w1b, "w2p": w2p,
                "wl1p": wl1p, "wl2t": wl2t, "vc": vc, "bl2c": bl2c,
                "ones": np.ones((128, 1), f32),
                "mones": np.full((1, 16), -1.0, f32),
            }
        )
    return in_maps


def kernel(**inputs):
    from concourse.bass_utils import run_bass_kernel_spmd

    nc = _get_nc()
    in_maps = make_in_maps(inputs)
    res = run_bass_kernel_spmd(nc, in_maps, core_ids=list(range(NCORES)))
    return np.concatenate(
        [np.asarray(r["out"], dtype=np.float32).T for r in res.results], axis=0
    )


# revision 11
# speedup vs baseline: 1.4079x; 1.0116x over previous
"""Trainium2 Bass kernel for ChebyNet (K=1) forward pass.

ChebConv with K=1 reduces to a plain linear layer on the T0 (identity) term,
so edge_index / edge_weight never enter the math. The network is:

    h1 = x @ W1.T (+b1)           -> BN (train mode, over nodes) -> ReLU
    h2 = a1 @ W2.T (+b2)          -> BN -> ReLU
    h3 = relu(h2 @ Wl1.T + bl1)
    out = log_softmax(h3 @ Wl2.T + bl2, axis=1)

(b1/b2 cancel exactly inside train-mode BN and are dropped.)

Sharding: nodes (N=50000) split across 8 NeuronCores (6250 rows each).
Everything is computed feature-on-partition ([feat, rows]).

Design (vs the 516-593us v0 baseline):
 - All transposes / dtype packing on host: xT, x row-tiles (with a ones
   column for the column-sum), W1T, W2T/Wl1T/Wl2T, per-partition BN
   parameter columns. No on-device transposes.
 - bf16 operands everywhere on the matmul path (fp8 DoubleRow measured at
   rel_err 2.6-3.4e-2 for the K=1024 layers - over the 2e-2 gate).
 - BN1 stats analytically from the Gram matrix of x, projected locally to
   diag(W1 G W1^T) BEFORE the AllReduce -> payload [128,16] (8KB).
 - During the AR1 wait, L1 (pre-BN) is computed for all chunks into SBUF
   (bf16). The main pass applies BN1+ReLU, runs L2, and overwrites h1 with
   h2 *in place* in SBUF - h2 never spills to DRAM.
 - BN2 stats: sum(h2) via vector tensor_scalar accum_out during the
   PSUM->SBUF copy; sumsq(h2) split scalar(Square+accum)/vector(STT+accum).
 - log_softmax batched at the end ([10, R] feature-major) so the scalar
   activation table is not thrashed per chunk (Relu<->Exp<->Ln reloads cost
   1.3us each); -ln(sum) is broadcast via a K=1 matmul. Output is [10, R];
   host transposes to [R, 10] (layout-only, like the xT input).
"""

import os
import sys

sys.path.insert(0, "/opt/trn_rl_repo")

import numpy as np
import ml_dtypes

NCORES = 8
N_TOTAL = 50000
R = N_TOTAL // NCORES  # 6250 rows per core
DIN = 128
H = 1024
HM = 256
C = 10
BN_EPS = 1e-5

CH = 1024  # main-pass row chunk
FCH = 512  # final-pass row chunk
CH_LIST = [(i * CH, min(CH, R - i * CH)) for i in range((R + CH - 1) // CH)]
FCH_LIST = [(i * FCH, min(FCH, R - i * FCH)) for i in range((R + FCH - 1) // FCH)]
if os.environ.get("CH_LIMIT"):
    CH_LIST = CH_LIST[: int(os.environ["CH_LIMIT"])]
NCH = len(CH_LIST)

NRT = (R + 127) // 128  # 49 row tiles for the Gram matrix
D1 = DIN + 1  # x tile width incl the ones column

L2_FP8 = os.environ.get("L2_FP8", "0") == "1"
L3_FP8 = os.environ.get("L3_FP8", "0") == "1"

_CACHE = {}


def _halves(cc, step=512):
    out = []
    off = 0
    while off < cc:
        out.append((off, min(step, cc - off)))
        off += step
    return out


def _build(stage="full"):
    import concourse.bass as bass  # noqa: F401
    import concourse.tile as tile
    import concourse.mybir as mybir
    from concourse import bacc

    fp32 = mybir.dt.float32
    f32r = mybir.dt.float32r
    bf16 = mybir.dt.bfloat16
    fp8 = mybir.dt.float8e4
    AF = mybir.ActivationFunctionType
    ALU = mybir.AluOpType
    X = mybir.AxisListType.X
    DR = mybir.MatmulPerfMode.DoubleRow

    l2dt = fp8 if L2_FP8 else bf16
    l3dt = fp8 if L3_FP8 else bf16

    nc = bacc.Bacc(num_devices=NCORES, debug=False)

    x2_d = nc.dram_tensor("x2", [128, NRT * D1], bf16, kind="ExternalInput")
    xt_d = nc.dram_tensor("xT", [128, R], bf16, kind="ExternalInput")
    w1f_d = nc.dram_tensor("w1f", [128, H], f32r, kind="ExternalInput")
    w1b_d = nc.dram_tensor("w1b", [128, H], bf16, kind="ExternalInput")
    w2p_d = nc.dram_tensor("w2p", [128, 8 * H], l2dt, kind="ExternalInput")
    wl1p_d = nc.dram_tensor("wl1p", [128, 8 * HM], l3dt, kind="ExternalInput")
    wl2t_d = nc.dram_tensor("wl2t", [128, 2 * C], bf16, kind="ExternalInput")
    vc_d = nc.dram_tensor("vc", [128, 64], fp32, kind="ExternalInput")
    bl2c_d = nc.dram_tensor("bl2c", [16, 1], fp32, kind="ExternalInput")
    ones_d = nc.dram_tensor("ones", [128, 1], f32r, kind="ExternalInput")
    mones_d = nc.dram_tensor("mones", [1, 16], f32r, kind="ExternalInput")
    out_d = nc.dram_tensor("out", [C, R], fp32, kind="ExternalOutput")

    RG = [list(range(NCORES))]

    with tile.TileContext(nc) as tc:
        with (
            tc.tile_pool(name="persist", bufs=1) as persist,
            tc.tile_pool(name="dram", bufs=1, space="DRAM") as dram,
        ):
            # ---------------- persistent tiles -----------------
            hbuf = [
                persist.tile([128, R], bf16, tag=f"hb{m}", name=f"hbuf{m}")
                for m in range(8)
            ]
            w1bs = persist.tile([128, H], bf16, tag="w1bs", name="w1bs")
            w2ps = persist.tile([128, 8 * H], l2dt, tag="w2ps", name="w2ps")
            wl1ps = persist.tile([128, 8 * HM], l3dt, tag="wl1ps", name="wl1ps")
            wl2ts = persist.tile([128, 2 * C], bf16, tag="wl2ts", name="wl2ts")
            vcs = persist.tile([128, 64], fp32, tag="vcs", name="vcs")
            bl2cs = persist.tile([16, 1], fp32, tag="bl2cs", name="bl2cs")
            ones_r = persist.tile([128, 1], f32r, tag="ones_r", name="ones_r")
            mones10 = persist.tile([1, 16], f32r, tag="mones", name="mones10")
            bn1_s = persist.tile([128, 8], fp32, tag="bn1s", name="bn1_s")
            bn1_b = persist.tile([128, 8], fp32, tag="bn1b", name="bn1_b")
            bn2_s = persist.tile([128, 8], fp32, tag="bn2s", name="bn2_s")
            bn2_b = persist.tile([128, 8], fp32, tag="bn2b", name="bn2_b")
            eps_c = persist.tile([128, 1], fp32, tag="epsc", name="eps_c")
            suma = persist.tile([128, 8 * NCH], fp32, tag="suma", name="suma")
            sumq = persist.tile([128, 8 * NCH], fp32, tag="sumq", name="sumq")
            st1g = persist.tile([128, 16], fp32, tag="st1g", name="st1g")
            st2g = persist.tile([128, 16], fp32, tag="st2g", name="st2g")
            ones_bf = persist.tile([128, 1], bf16, tag="onesbf", name="ones_bf")
            mones_bf = persist.tile([1, 16], bf16, tag="monesbf", name="mones_bf")
            lgall = persist.tile([16, R], bf16, tag="lgall", name="lgall")
            e_all = persist.tile([16, R], bf16, tag="e_all", name="e_all")

            cc1_in = dram.tile([128, 16], fp32, name="cc1_in")
            cc1_out = dram.tile([128, 16], fp32, name="cc1_out")
            cc2_in = dram.tile([128, 16], fp32, name="cc2_in")
            cc2_out = dram.tile([128, 16], fp32, name="cc2_out")

            nc.vector.memset(eps_c[:], BN_EPS)
            nc.vector.memset(ones_bf[:], 1.0)
            nc.vector.memset(mones_bf[:], -1.0)
            nc.scalar.dma_start(out=ones_r[:], in_=ones_d[:])
            nc.scalar.dma_start(out=mones10[:], in_=mones_d[:])

            # per-partition views of the BN parameter columns
            vcv = vcs[:].rearrange("p (m j) -> p j m", j=8)  # [128, j, m]

            # ============ startup + prefill (xT lives only here) ============
            with tc.tile_pool(name="xtp", bufs=1) as xtp:
                xTs = xtp.tile([128, R], bf16, tag="xTs", name="xTs")

                with (
                    tc.tile_pool(name="startsb", bufs=1) as startsb,
                    tc.tile_pool(name="startps", bufs=1, space="PSUM") as startps,
                ):
                    x2s = startsb.tile(
                        [128, NRT * D1], bf16, tag="x2s", name="x2s"
                    )
                    w1fr = startsb.tile([128, H], f32r, tag="w1fr", name="w1fr")
                    v_r = startsb.tile([128, H], bf16, tag="v_r", name="v_r")
                    mean_r = startsb.tile([128, 1], bf16, tag="mean_r", name="mean_r")
                    st1sb = startsb.tile([128, 16], fp32, tag="st1sb", name="st1sb")

                    # big loads on the sync DMA queue; small ones on scalar
                    nc.sync.dma_start(out=x2s[:], in_=x2_d[:])
                    nc.sync.dma_start(out=xTs[:], in_=xt_d[:])
                    nc.sync.dma_start(out=w2ps[:], in_=w2p_d[:])
                    nc.sync.dma_start(out=wl1ps[:], in_=wl1p_d[:])
                    nc.scalar.dma_start(out=w1fr[:], in_=w1f_d[:])
                    nc.scalar.dma_start(out=w1bs[:], in_=w1b_d[:])
                    nc.scalar.dma_start(out=vcs[:], in_=vc_d[:])
                    nc.scalar.dma_start(out=wl2ts[:], in_=wl2t_d[:])
                    nc.scalar.dma_start(out=bl2cs[:], in_=bl2c_d[:])

                    # Gram matrix of x (incl ones column -> column sums)
                    gram_ps = startps.tile(
                        [128, D1], fp32, tag="gram", name="gram_ps"
                    )
                    for t in range(NRT):
                        o = t * D1
                        nc.tensor.matmul(
                            gram_ps[:],
                            lhsT=x2s[:, o : o + DIN],
                            rhs=x2s[:, o : o + D1],
                            start=(t == 0),
                            stop=(t == NRT - 1),
                        )
                    gram_r = startsb.tile(
                        [128, D1], f32r, tag="gram_r", name="gram_r"
                    )
                    nc.vector.tensor_copy(gram_r[:], gram_ps[:])
                    nc.scalar.mul(mean_r[:], gram_r[:, DIN : D1], 1.0 / N_TOTAL)

                    # P = G @ W1T ; V = W1T*P ; e2[f]=colsum(V) ; wxm = W1T.T mean
                    st1_ps = startps.tile(
                        [128, 16], fp32, tag="st1ps", name="st1_ps"
                    )
                    for hf in range(2):
                        sl = slice(hf * 512, (hf + 1) * 512)
                        p_ps = startps.tile(
                            [128, 512], fp32, tag=f"pps{hf}", name=f"p_ps{hf}"
                        )
                        nc.tensor.matmul(
                            p_ps[:], lhsT=gram_r[:, 0:DIN], rhs=w1fr[:, sl],
                            start=True, stop=True,
                        )
                        nc.vector.tensor_mul(v_r[:, sl], w1fr[:, sl], p_ps[:])
                    for m in range(8):
                        sl = slice(m * 128, (m + 1) * 128)
                        nc.tensor.matmul(
                            st1_ps[:, m : m + 1], lhsT=v_r[:, sl], rhs=ones_bf[:],
                            start=True, stop=True,
                        )
                        nc.tensor.matmul(
                            st1_ps[:, 8 + m : 9 + m], lhsT=w1bs[:, sl],
                            rhs=mean_r[:], start=True, stop=True,
                        )
                    nc.vector.tensor_copy(st1sb[:], st1_ps[:])
                    nc.sync.dma_start(out=cc1_in[:], in_=st1sb[:])
                    nc.gpsimd.collective_compute(
                        "AllReduce", ALU.add, replica_groups=RG,
                        ins=[cc1_in[:].opt()], outs=[cc1_out[:].opt()],
                    )
                    nc.sync.dma_start(out=st1g[:], in_=cc1_out[:])

                # -------- L1 prefill (runs during the AllReduce wait) -------
                with tc.tile_pool(name="ph1", bufs=4, space="PSUM") as ph1:
                    for ci, (c0, cc) in enumerate(CH_LIST):
                        for m in range(8):
                            hp = ph1.tile(
                                [128, CH], fp32, tag="ph1", name=f"h1_{ci}_{m}"
                            )
                            for off, nn in _halves(cc):
                                nc.tensor.matmul(
                                    hp[:, off : off + nn],
                                    lhsT=w1bs[:, m * 128 : (m + 1) * 128],
                                    rhs=xTs[:, c0 + off : c0 + off + nn],
                                    start=True,
                                    stop=True,
                                )
                            if m % 2 == 0:
                                nc.scalar.copy(
                                    hbuf[m][:, c0 : c0 + cc], hp[:, :cc]
                                )
                            else:
                                nc.vector.tensor_copy(
                                    hbuf[m][:, c0 : c0 + cc], hp[:, :cc]
                                )

            # ---------------- BN1 parameters (vectorized) ----------------
            with tc.tile_pool(name="bnw", bufs=1) as bnw:
                var8 = bnw.tile([128, 8], fp32, tag="v8", name="var8")
                msq8 = bnw.tile([128, 8], fp32, tag="m8", name="msq8")
                sd8 = bnw.tile([128, 8], fp32, tag="s8", name="sd8")
                rstd8 = bnw.tile([128, 8], fp32, tag="r8", name="rstd8")
                t8 = bnw.tile([128, 8], fp32, tag="t8", name="t8")
                nc.vector.tensor_scalar_mul(var8[:], st1g[:, 0:8], 1.0 / N_TOTAL)
                nc.vector.tensor_mul(msq8[:], st1g[:, 8:16], st1g[:, 8:16])
                nc.vector.tensor_sub(var8[:], var8[:], msq8[:])
                nc.scalar.activation(sd8[:], var8[:], AF.Sqrt, bias=eps_c[:])
                nc.vector.reciprocal(rstd8[:], sd8[:])
                nc.vector.tensor_mul(bn1_s[:], rstd8[:], vcv[:, 1, :])
                nc.vector.tensor_mul(t8[:], st1g[:, 8:16], bn1_s[:])
                nc.vector.tensor_sub(bn1_b[:], vcv[:, 2, :], t8[:])

            if stage == "s1":
                dummy = persist.tile([16, R], fp32, tag="dummy", name="dummy")
                nc.vector.memset(dummy[:], 0.0)
                nc.vector.tensor_copy(dummy[:10, 0:8], bn1_s[:10, :])
                nc.vector.tensor_copy(dummy[:10, 8:16], bn1_b[:10, :])
                nc.sync.dma_start(out=out_d[:], in_=dummy[:10, :])
                nc.finalize()
                return nc

            # ------------- main pass: BN1+ReLU -> L2 -> stats ------------
            w2v = w2ps[:].rearrange("p (kp two h) -> p kp two h", kp=4, two=2)
            with (
                tc.tile_pool(name="acts", bufs=1) as acts,
                tc.tile_pool(name="sqp", bufs=1) as sqp,
                tc.tile_pool(name="ph2", bufs=2, space="PSUM") as ph2,
            ):
                for ci, (c0, cc) in enumerate(CH_LIST):
                    sl = slice(c0, c0 + cc)
                    a1 = acts.tile(
                        [128, 8 * CH], l2dt, tag=f"act{ci & 1}", name=f"a1_{ci}"
                    )
                    a1v = a1[:].rearrange("p (k c) -> p k c", k=8)
                    for m in range(8):
                        nc.scalar.activation(
                            a1[:, m * CH : m * CH + cc],
                            hbuf[m][:, sl],
                            AF.Relu,
                            bias=bn1_b[:, m : m + 1],
                            scale=bn1_s[:, m : m + 1],
                        )
                    for m2 in range(8):
                        hp = ph2.tile([128, CH], fp32, tag="ph2", name=f"h2_{ci}_{m2}")
                        msl = slice(m2 * 128, (m2 + 1) * 128)
                        if L2_FP8:
                            for off, nn in _halves(cc):
                                for kp in range(4):
                                    nc.tensor.matmul(
                                        hp[:, off : off + nn],
                                        lhsT=w2v[:, kp, :, msl],
                                        rhs=a1v[:, 2 * kp : 2 * kp + 2, off : off + nn],
                                        start=(kp == 0),
                                        stop=(kp == 3),
                                        perf_mode=DR,
                                    )
                        else:
                            for k in range(8):
                                for off, nn in _halves(cc):
                                    nc.tensor.matmul(
                                        hp[:, off : off + nn],
                                        lhsT=w2v[:, k // 2, k % 2, msl],
                                        rhs=a1v[:, k, off : off + nn],
                                        start=(k == 0),
                                        stop=(k == 7),
                                    )
                        # PSUM -> SBUF (bf16, in place over h1) + row-sum
                        nc.vector.tensor_scalar(
                            out=hbuf[m2][:, sl],
                            in0=hp[:, :cc],
                            scalar1=1.0,
                            scalar2=0.0,
                            op0=ALU.mult,
                            op1=ALU.add,
                            accum_out=suma[:, m2 * NCH + ci : m2 * NCH + ci + 1],
                        )
                        sq = sqp.tile(
                            [128, CH], bf16, tag=f"sq{m2 & 1}", name=f"sq_{ci}_{m2}"
                        )
                        if m2 < 3:
                            nc.scalar.activation(
                                sq[:, :cc],
                                hbuf[m2][:, sl],
                                AF.Square,
                                accum_out=sumq[:, m2 * NCH + ci : m2 * NCH + ci + 1],
                            )
                        else:
                            nc.vector.scalar_tensor_tensor(
                                out=sq[:, :cc],
                                in0=hbuf[m2][:, sl],
                                scalar=1.0,
                                in1=hbuf[m2][:, sl],
                                op0=ALU.mult,
                                op1=ALU.mult,
                                accum_out=sumq[:, m2 * NCH + ci : m2 * NCH + ci + 1],
                            )

                # ---------------- BN2 statistics ----------------
                st2sb = sqp.tile([128, 16], fp32, tag="st2sb", name="st2sb")
                sumav = suma[:].rearrange("p (m c) -> p m c", m=8)
                sumqv = sumq[:].rearrange("p (m c) -> p m c", m=8)
                nc.vector.reduce_sum(st2sb[:, 0:8], sumav, axis=X)
                nc.vector.reduce_sum(st2sb[:, 8:16], sumqv, axis=X)
                nc.sync.dma_start(out=cc2_in[:], in_=st2sb[:])
                nc.gpsimd.collective_compute(
                    "AllReduce", ALU.add, replica_groups=RG,
                    ins=[cc2_in[:].opt()], outs=[cc2_out[:].opt()],
                )
                nc.sync.dma_start(out=st2g[:], in_=cc2_out[:])

            # ---------------- BN2 parameters (vectorized) ----------------
            with tc.tile_pool(name="bnw2", bufs=1) as bnw2:
                mean8 = bnw2.tile([128, 8], fp32, tag="n8", name="mean8")
                var8b = bnw2.tile([128, 8], fp32, tag="v8", name="var8b")
                msq8b = bnw2.tile([128, 8], fp32, tag="m8", name="msq8b")
                sd8b = bnw2.tile([128, 8], fp32, tag="s8", name="sd8b")
                rstd8b = bnw2.tile([128, 8], fp32, tag="r8", name="rstd8b")
                t8b = bnw2.tile([128, 8], fp32, tag="t8", name="t8b")
                nc.vector.tensor_scalar_mul(mean8[:], st2g[:, 0:8], 1.0 / N_TOTAL)
                nc.vector.tensor_scalar_mul(var8b[:], st2g[:, 8:16], 1.0 / N_TOTAL)
                nc.vector.tensor_mul(msq8b[:], mean8[:], mean8[:])
                nc.vector.tensor_sub(var8b[:], var8b[:], msq8b[:])
                nc.scalar.activation(sd8b[:], var8b[:], AF.Sqrt, bias=eps_c[:])
                nc.vector.reciprocal(rstd8b[:], sd8b[:])
                nc.vector.tensor_mul(bn2_s[:], rstd8b[:], vcv[:, 4, :])
                nc.vector.tensor_mul(t8b[:], mean8[:], bn2_s[:])
                nc.vector.tensor_sub(bn2_b[:], vcv[:, 5, :], t8b[:])

            # ------ final pass A: BN2+ReLU -> L3 -> L4 -> logits ------
            wl1v = wl1ps[:].rearrange("p (kp two h) -> p kp two h", kp=4, two=2)
            with (
                tc.tile_pool(name="acts2", bufs=1) as acts2,
                tc.tile_pool(name="fsb", bufs=2) as fsb,
                tc.tile_pool(name="ph3", bufs=2, space="PSUM") as ph3,
                tc.tile_pool(name="plg", bufs=2, space="PSUM") as plg,
            ):
                for ci, (c0, cc) in enumerate(FCH_LIST):
                    sl = slice(c0, c0 + cc)
                    a2 = acts2.tile(
                        [128, 8 * FCH], l3dt, tag=f"act{ci & 1}", name=f"a2_{ci}"
                    )
                    a2v = a2[:].rearrange("p (k c) -> p k c", k=8)
                    for m in range(8):
                        if m < 6:
                            nc.scalar.activation(
                                a2[:, m * FCH : m * FCH + cc],
                                hbuf[m][:, sl],
                                AF.Relu,
                                bias=bn2_b[:, m : m + 1],
                                scale=bn2_s[:, m : m + 1],
                            )
                        else:
                            tmp = fsb.tile(
                                [128, FCH], bf16, tag=f"tmp{m}", name=f"tmp_{ci}_{m}"
                            )
                            nc.vector.tensor_scalar(
                                out=tmp[:, :cc],
                                in0=hbuf[m][:, sl],
                                scalar1=bn2_s[:, m : m + 1],
                                scalar2=bn2_b[:, m : m + 1],
                                op0=ALU.mult,
                                op1=ALU.add,
                            )
                            nc.vector.tensor_scalar_max(
                                a2[:, m * FCH : m * FCH + cc], tmp[:, :cc], 0.0
                            )
                    h3t = []
                    for m3 in range(2):
                        hp3 = ph3.tile(
                            [128, FCH], fp32, tag="ph3", name=f"h3_{ci}_{m3}"
                        )
                        msl = slice(m3 * 128, (m3 + 1) * 128)
                        if L3_FP8:
                            for kp in range(4):
                                nc.tensor.matmul(
                                    hp3[:, :cc],
                                    lhsT=wl1v[:, kp, :, msl],
                                    rhs=a2v[:, 2 * kp : 2 * kp + 2, :cc],
                                    start=(kp == 0),
                                    stop=(kp == 3),
                                    perf_mode=DR,
                                )
                        else:
                            for k in range(8):
                                nc.tensor.matmul(
                                    hp3[:, :cc],
                                    lhsT=wl1v[:, k // 2, k % 2, msl],
                                    rhs=a2v[:, k, :cc],
                                    start=(k == 0),
                                    stop=(k == 7),
                                )
                        h3 = fsb.tile(
                            [128, FCH], bf16, tag=f"h3_{m3}", name=f"h3t_{ci}_{m3}"
                        )
                        nc.vector.tensor_scalar(
                            out=h3[:, :cc],
                            in0=hp3[:, :cc],
                            scalar1=vcv[:, 6, m3 : m3 + 1],
                            scalar2=0.0,
                            op0=ALU.add,
                            op1=ALU.max,
                        )
                        h3t.append(h3)
                    # L4: logits (pre-bl2) -> PSUM -> lgall (bf16)
                    lg = plg.tile([C, FCH], fp32, tag="lg", name=f"lg_{ci}")
                    for k in range(2):
                        nc.tensor.matmul(
                            lg[:, :cc],
                            lhsT=wl2ts[:, k * C : (k + 1) * C],
                            rhs=h3t[k][:, :cc],
                            start=(k == 0),
                            stop=(k == 1),
                        )
                    nc.vector.tensor_copy(lgall[:C, sl], lg[:, :cc])

            # ---------- batched log_softmax over [10, R] ----------
            with (
                tc.tile_pool(name="smx", bufs=2) as smx,
                tc.tile_pool(name="psmB", bufs=2, space="PSUM") as psmB,
            ):
                EW = 2048
                for o0 in range(0, R, EW):
                    ee = min(EW, R - o0)
                    nc.scalar.activation(
                        e_all[:C, o0 : o0 + ee],
                        lgall[:C, o0 : o0 + ee],
                        AF.Exp,
                        bias=bl2cs[:C, :],
                    )
                for ci, (c0, cc) in enumerate(FCH_LIST):
                    sl = slice(c0, c0 + cc)
                    sm = psmB.tile([1, FCH], fp32, tag="sm", name=f"smB_{ci}")
                    nc.tensor.matmul(
                        sm[:, :cc],
                        lhsT=ones_bf[:C, :],
                        rhs=e_all[:C, sl],
                        start=True,
                        stop=True,
                    )
                    lns = smx.tile([1, FCH], bf16, tag="lns", name=f"lnsB_{ci}")
                    nc.scalar.activation(lns[:, :cc], sm[:, :cc], AF.Ln)
                    bc = psmB.tile([C, FCH], fp32, tag="bc", name=f"bcB_{ci}")
                    nc.tensor.matmul(
                        bc[:, :cc],
                        lhsT=mones_bf[:, :C],
                        rhs=lns[:, :cc],
                        start=True,
                        stop=True,
                    )
                    osb = smx.tile([C, FCH], fp32, tag="osb", name=f"osbB_{ci}")
                    nc.vector.scalar_tensor_tensor(
                        out=osb[:, :cc],
                        in0=lgall[:C, sl],
                        scalar=bl2cs[:C, :],
                        in1=bc[:, :cc],
                        op0=ALU.add,
                        op1=ALU.add,
                    )
                    nc.sync.dma_start(out=out_d[:, sl], in_=osb[:, :cc])

    nc.finalize()
    return nc


def _get_nc():
    if "nc" not in _CACHE:
        _CACHE["nc"] = _build(os.environ.get("KERNEL_STAGE", "full"))
    return _CACHE["nc"]


def make_in_maps(inputs):
    """Host-side layout/dtype prep. Layout-only transforms (transpose, tile,
    pack, cast) - all math happens on device."""
    f32 = np.float32
    bf = ml_dtypes.bfloat16
    f8 = ml_dtypes.float8_e4m3

    x = np.ascontiguousarray(np.asarray(inputs["x"]), dtype=f32)
    W1 = np.asarray(inputs["W1"], dtype=f32)
    W2 = np.asarray(inputs["W2"], dtype=f32)
    Wl1 = np.asarray(inputs["Wl1"], dtype=f32)
    Wl2 = np.asarray(inputs["Wl2"], dtype=f32)

    w1f = np.ascontiguousarray(W1.T)  # [128, H] f32 (f32r on device)
    w1b = w1f.astype(bf)
    l2np = f8 if L2_FP8 else bf
    l3np = f8 if L3_FP8 else bf
    # [128, kp, two, m] packing (DoubleRow-compatible; plain reshape for bf16)
    w2p = np.ascontiguousarray(
        W2.T.reshape(4, 2, 128, H).transpose(2, 0, 1, 3).reshape(128, 8 * H)
    ).astype(l2np)
    wl1p = np.ascontiguousarray(
        Wl1.T.reshape(4, 2, 128, HM).transpose(2, 0, 1, 3).reshape(128, 8 * HM)
    ).astype(l3np)
    wl2t = np.ascontiguousarray(
        Wl2.T.reshape(2, 128, C).transpose(1, 0, 2).reshape(128, 2 * C)
    ).astype(bf)

    vecs = np.zeros((8, H), f32)
    vecs[1, :] = np.asarray(inputs["g1"], dtype=f32)
    vecs[2, :] = np.asarray(inputs["be1"], dtype=f32)
    vecs[4, :] = np.asarray(inputs["g2"], dtype=f32)
    vecs[5, :] = np.asarray(inputs["be2"], dtype=f32)
    vecs[6, :HM] = np.asarray(inputs["bl1"], dtype=f32)
    # vc[p, k, j] = vecs[j, k*128+p]
    vc = np.ascontiguousarray(
        vecs.T.reshape(8, 128, 8).transpose(1, 0, 2).reshape(128, 64)
    )
    bl2c = np.zeros((16, 1), f32)
    bl2c[:C, 0] = np.asarray(inputs["bl2"], dtype=f32)

    in_maps = []
    for i in range(NCORES):
        xs = x[i * R : (i + 1) * R]
        xt = np.ascontiguousarray(xs.T).astype(bf)
        x2 = np.zeros((NRT * 128, D1), f32)
        x2[:R, :DIN] = xs
        x2[:R, DIN] = 1.0
        x2 = np.ascontiguousarray(
            x2.reshape(NRT, 128, D1).transpose(1, 0, 2).reshape(128, NRT * D1)
        ).astype(bf)
        in_maps.append(
            {
                "x2": x2, "xT": xt, "w1f": w1f, "w1b": w1b, "w2p": w2p,
                "wl1p": wl1p, "wl2t": wl2t, "vc": vc, "bl2c": bl2c,
                "ones": np.ones((128, 1), f32),
                "mones": np.full((1, 16), -1.0, f32),
            }
        )
    return in_maps


def kernel(**inputs):
    from concourse.bass_utils import run_bass_kernel_spmd

    nc = _get_nc()
    in_maps = make_in_maps(inputs)
    res = run_bass_kernel_spmd(nc, in_maps, core_ids=list(range(NCORES)))
    return np.concatenate(
        [np.asarray(r["out"], dtype=np.float32).T for r in res.results], axis=0
    )


# revision 12
# speedup vs baseline: 1.4517x; 1.0311x over previous
"""Trainium2 Bass kernel for ChebyNet (K=1) forward pass.

ChebConv with K=1 reduces to a plain linear layer on the T0 (identity) term,
so edge_index / edge_weight never enter the math. The network is:

    h1 = x @ W1.T (+b1)           -> BN (train mode, over nodes) -> ReLU
    h2 = a1 @ W2.T (+b2)          -> BN -> ReLU
    h3 = relu(h2 @ Wl1.T + bl1)
    out = log_softmax(h3 @ Wl2.T + bl2, axis=1)

(b1/b2 cancel exactly inside train-mode BN and are dropped.)

Sharding: nodes (N=50000) split across 8 NeuronCores (6250 rows each).
Everything is computed feature-on-partition ([feat, rows]).

Design (vs the 516-593us v0 baseline):
 - All transposes / dtype packing on host: xT, x row-tiles (with a ones
   column for the column-sum), W1T, W2T/Wl1T/Wl2T, per-partition BN
   parameter columns. No on-device transposes.
 - bf16 operands everywhere on the matmul path (fp8 DoubleRow measured at
   rel_err 2.6-3.4e-2 for the K=1024 layers - over the 2e-2 gate).
 - BN1 stats analytically from the Gram matrix of x, projected locally to
   diag(W1 G W1^T) BEFORE the AllReduce -> payload [128,16] (8KB).
 - During the AR1 wait, L1 (pre-BN) is computed for all chunks into SBUF
   (bf16). The main pass applies BN1+ReLU, runs L2, and overwrites h1 with
   h2 *in place* in SBUF - h2 never spills to DRAM.
 - BN2 stats: sum(h2) via vector tensor_scalar accum_out during the
   PSUM->SBUF copy; sumsq(h2) split scalar(Square+accum)/vector(STT+accum).
 - log_softmax batched at the end ([10, R] feature-major) so the scalar
   activation table is not thrashed per chunk (Relu<->Exp<->Ln reloads cost
   1.3us each); -ln(sum) is broadcast via a K=1 matmul. Output is [10, R];
   host transposes to [R, 10] (layout-only, like the xT input).
"""

import os
import sys

sys.path.insert(0, "/opt/trn_rl_repo")

import numpy as np
import ml_dtypes

NCORES = 8
N_TOTAL = 50000
R = N_TOTAL // NCORES  # 6250 rows per core
DIN = 128
H = 1024
HM = 256
C = 10
BN_EPS = 1e-5

CH = 1024  # main-pass row chunk
FCH = 512  # final-pass row chunk
CH_LIST = [(i * CH, min(CH, R - i * CH)) for i in range((R + CH - 1) // CH)]
FCH_LIST = [(i * FCH, min(FCH, R - i * FCH)) for i in range((R + FCH - 1) // FCH)]
if os.environ.get("CH_LIMIT"):
    CH_LIST = CH_LIST[: int(os.environ["CH_LIMIT"])]
NCH = len(CH_LIST)

NRT = (R + 127) // 128  # 49 row tiles for the Gram matrix
D1 = DIN + 1  # x tile width incl the ones column

L2_FP8 = os.environ.get("L2_FP8", "0") == "1"
L3_FP8 = os.environ.get("L3_FP8", "0") == "1"

_CACHE = {}


def _halves(cc, step=512):
    out = []
    off = 0
    while off < cc:
        out.append((off, min(step, cc - off)))
        off += step
    return out


def _build(stage="full"):
    import concourse.bass as bass  # noqa: F401
    import concourse.tile as tile
    import concourse.mybir as mybir
    from concourse import bacc

    fp32 = mybir.dt.float32
    f32r = mybir.dt.float32r
    bf16 = mybir.dt.bfloat16
    fp8 = mybir.dt.float8e4
    AF = mybir.ActivationFunctionType
    ALU = mybir.AluOpType
    X = mybir.AxisListType.X
    DR = mybir.MatmulPerfMode.DoubleRow

    l2dt = fp8 if L2_FP8 else bf16
    l3dt = fp8 if L3_FP8 else bf16

    nc = bacc.Bacc(num_devices=NCORES, debug=False)

    x2_d = nc.dram_tensor("x2", [128, NRT * D1], bf16, kind="ExternalInput")
    xt_d = nc.dram_tensor("xT", [128, R], bf16, kind="ExternalInput")
    w1f_d = nc.dram_tensor("w1f", [128, H], f32r, kind="ExternalInput")
    w1b_d = nc.dram_tensor("w1b", [128, H], bf16, kind="ExternalInput")
    w2p_d = nc.dram_tensor("w2p", [128, 8 * H], l2dt, kind="ExternalInput")
    wl1p_d = nc.dram_tensor("wl1p", [128, 8 * HM], l3dt, kind="ExternalInput")
    wl2t_d = nc.dram_tensor("wl2t", [128, 2 * C], bf16, kind="ExternalInput")
    vc_d = nc.dram_tensor("vc", [128, 64], fp32, kind="ExternalInput")
    bl2c_d = nc.dram_tensor("bl2c", [16, 1], fp32, kind="ExternalInput")
    ones_d = nc.dram_tensor("ones", [128, 1], f32r, kind="ExternalInput")
    mones_d = nc.dram_tensor("mones", [1, 16], f32r, kind="ExternalInput")
    out_d = nc.dram_tensor("out", [C, R], fp32, kind="ExternalOutput")

    RG = [list(range(NCORES))]

    with tile.TileContext(nc) as tc:
        with (
            tc.tile_pool(name="persist", bufs=1) as persist,
            tc.tile_pool(name="dram", bufs=1, space="DRAM") as dram,
        ):
            # ---------------- persistent tiles -----------------
            hbuf = [
                persist.tile([128, R], bf16, tag=f"hb{m}", name=f"hbuf{m}")
                for m in range(8)
            ]
            w1bs = persist.tile([128, H], bf16, tag="w1bs", name="w1bs")
            w2ps = persist.tile([128, 8 * H], l2dt, tag="w2ps", name="w2ps")
            wl1ps = persist.tile([128, 8 * HM], l3dt, tag="wl1ps", name="wl1ps")
            wl2ts = persist.tile([128, 2 * C], bf16, tag="wl2ts", name="wl2ts")
            vcs = persist.tile([128, 64], fp32, tag="vcs", name="vcs")
            bl2cs = persist.tile([16, 1], fp32, tag="bl2cs", name="bl2cs")
            ones_r = persist.tile([128, 1], f32r, tag="ones_r", name="ones_r")
            mones10 = persist.tile([1, 16], f32r, tag="mones", name="mones10")
            bn1_s = persist.tile([128, 8], fp32, tag="bn1s", name="bn1_s")
            bn1_b = persist.tile([128, 8], fp32, tag="bn1b", name="bn1_b")
            bn2_s = persist.tile([128, 8], fp32, tag="bn2s", name="bn2_s")
            bn2_b = persist.tile([128, 8], fp32, tag="bn2b", name="bn2_b")
            eps_c = persist.tile([128, 1], fp32, tag="epsc", name="eps_c")
            suma = persist.tile([128, 8 * NCH], fp32, tag="suma", name="suma")
            sumq = persist.tile([128, 8 * NCH], fp32, tag="sumq", name="sumq")
            st1g = persist.tile([128, 16], fp32, tag="st1g", name="st1g")
            st2g = persist.tile([128, 16], fp32, tag="st2g", name="st2g")
            ones_bf = persist.tile([128, 1], bf16, tag="onesbf", name="ones_bf")
            mones_bf = persist.tile([1, 16], bf16, tag="monesbf", name="mones_bf")
            lgall = persist.tile([16, R], bf16, tag="lgall", name="lgall")
            e_all = persist.tile([16, R], bf16, tag="e_all", name="e_all")

            cc1_in = dram.tile([128, 16], fp32, name="cc1_in")
            cc1_out = dram.tile([128, 16], fp32, name="cc1_out")
            cc2_in = dram.tile([128, 16], fp32, name="cc2_in")
            cc2_out = dram.tile([128, 16], fp32, name="cc2_out")

            nc.vector.memset(eps_c[:], BN_EPS)
            nc.vector.memset(ones_bf[:], 1.0)
            nc.vector.memset(mones_bf[:], -1.0)
            nc.scalar.dma_start(out=ones_r[:], in_=ones_d[:])
            nc.scalar.dma_start(out=mones10[:], in_=mones_d[:])

            # per-partition views of the BN parameter columns
            vcv = vcs[:].rearrange("p (m j) -> p j m", j=8)  # [128, j, m]

            # ============ startup + prefill (xT lives only here) ============
            with tc.tile_pool(name="xtp", bufs=1) as xtp:
                xTs = xtp.tile([128, R], bf16, tag="xTs", name="xTs")

                with (
                    tc.tile_pool(name="startsb", bufs=1) as startsb,
                    tc.tile_pool(name="startps", bufs=1, space="PSUM") as startps,
                ):
                    x2s = startsb.tile(
                        [128, NRT * D1], bf16, tag="x2s", name="x2s"
                    )
                    w1fr = startsb.tile([128, H], f32r, tag="w1fr", name="w1fr")
                    v_r = startsb.tile([128, H], bf16, tag="v_r", name="v_r")
                    mean_r = startsb.tile([128, 1], bf16, tag="mean_r", name="mean_r")
                    st1sb = startsb.tile([128, 16], fp32, tag="st1sb", name="st1sb")

                    # big loads on the sync DMA queue; small ones on scalar
                    nc.sync.dma_start(out=x2s[:], in_=x2_d[:])
                    nc.sync.dma_start(out=xTs[:], in_=xt_d[:])
                    nc.sync.dma_start(out=w2ps[:], in_=w2p_d[:])
                    nc.sync.dma_start(out=wl1ps[:], in_=wl1p_d[:])
                    nc.scalar.dma_start(out=w1fr[:], in_=w1f_d[:])
                    nc.scalar.dma_start(out=w1bs[:], in_=w1b_d[:])
                    nc.scalar.dma_start(out=vcs[:], in_=vc_d[:])
                    nc.scalar.dma_start(out=wl2ts[:], in_=wl2t_d[:])
                    nc.scalar.dma_start(out=bl2cs[:], in_=bl2c_d[:])

                    # Gram matrix of x (incl ones column -> column sums)
                    gram_ps = startps.tile(
                        [128, D1], fp32, tag="gram", name="gram_ps"
                    )
                    for t in range(NRT):
                        o = t * D1
                        nc.tensor.matmul(
                            gram_ps[:],
                            lhsT=x2s[:, o : o + DIN],
                            rhs=x2s[:, o : o + D1],
                            start=(t == 0),
                            stop=(t == NRT - 1),
                        )
                    gram_r = startsb.tile(
                        [128, D1], f32r, tag="gram_r", name="gram_r"
                    )
                    nc.vector.tensor_copy(gram_r[:], gram_ps[:])
                    nc.scalar.mul(mean_r[:], gram_r[:, DIN : D1], 1.0 / N_TOTAL)

                    # P = G @ W1T ; V = W1T*P ; e2[f]=colsum(V) ; wxm = W1T.T mean
                    st1_ps = startps.tile(
                        [128, 16], fp32, tag="st1ps", name="st1_ps"
                    )
                    for hf in range(2):
                        sl = slice(hf * 512, (hf + 1) * 512)
                        p_ps = startps.tile(
                            [128, 512], fp32, tag=f"pps{hf}", name=f"p_ps{hf}"
                        )
                        nc.tensor.matmul(
                            p_ps[:], lhsT=gram_r[:, 0:DIN], rhs=w1fr[:, sl],
                            start=True, stop=True,
                        )
                        nc.vector.tensor_mul(v_r[:, sl], w1fr[:, sl], p_ps[:])
                    for m in range(8):
                        sl = slice(m * 128, (m + 1) * 128)
                        nc.tensor.matmul(
                            st1_ps[:, m : m + 1], lhsT=v_r[:, sl], rhs=ones_bf[:],
                            start=True, stop=True,
                        )
                        nc.tensor.matmul(
                            st1_ps[:, 8 + m : 9 + m], lhsT=w1bs[:, sl],
                            rhs=mean_r[:], start=True, stop=True,
                        )
                    nc.vector.tensor_copy(st1sb[:], st1_ps[:])
                    nc.sync.dma_start(out=cc1_in[:], in_=st1sb[:])
                    nc.gpsimd.collective_compute(
                        "AllReduce", ALU.add, replica_groups=RG,
                        ins=[cc1_in[:].opt()], outs=[cc1_out[:].opt()],
                    )
                    nc.sync.dma_start(out=st1g[:], in_=cc1_out[:])

                # -------- L1 prefill (runs during the AllReduce wait) -------
                with tc.tile_pool(name="ph1", bufs=4, space="PSUM") as ph1:
                    for ci, (c0, cc) in enumerate(CH_LIST):
                        for m in range(8):
                            hp = ph1.tile(
                                [128, CH], fp32, tag="ph1", name=f"h1_{ci}_{m}"
                            )
                            for off, nn in _halves(cc):
                                nc.tensor.matmul(
                                    hp[:, off : off + nn],
                                    lhsT=w1bs[:, m * 128 : (m + 1) * 128],
                                    rhs=xTs[:, c0 + off : c0 + off + nn],
                                    start=True,
                                    stop=True,
                                )
                            if m % 2 == 0:
                                nc.scalar.copy(
                                    hbuf[m][:, c0 : c0 + cc], hp[:, :cc]
                                )
                            else:
                                nc.vector.tensor_copy(
                                    hbuf[m][:, c0 : c0 + cc], hp[:, :cc]
                                )

            # ---------------- BN1 parameters (vectorized) ----------------
            with tc.tile_pool(name="bnw", bufs=1) as bnw:
                var8 = bnw.tile([128, 8], fp32, tag="v8", name="var8")
                msq8 = bnw.tile([128, 8], fp32, tag="m8", name="msq8")
                sd8 = bnw.tile([128, 8], fp32, tag="s8", name="sd8")
                rstd8 = bnw.tile([128, 8], fp32, tag="r8", name="rstd8")
                t8 = bnw.tile([128, 8], fp32, tag="t8", name="t8")
                nc.vector.tensor_scalar_mul(var8[:], st1g[:, 0:8], 1.0 / N_TOTAL)
                nc.vector.tensor_mul(msq8[:], st1g[:, 8:16], st1g[:, 8:16])
                nc.vector.tensor_sub(var8[:], var8[:], msq8[:])
                nc.scalar.activation(sd8[:], var8[:], AF.Sqrt, bias=eps_c[:])
                nc.vector.reciprocal(rstd8[:], sd8[:])
                nc.vector.tensor_mul(bn1_s[:], rstd8[:], vcv[:, 1, :])
                nc.vector.tensor_mul(t8[:], st1g[:, 8:16], bn1_s[:])
                nc.vector.tensor_sub(bn1_b[:], vcv[:, 2, :], t8[:])

            if stage == "s1":
                dummy = persist.tile([16, R], fp32, tag="dummy", name="dummy")
                nc.vector.memset(dummy[:], 0.0)
                nc.vector.tensor_copy(dummy[:10, 0:8], bn1_s[:10, :])
                nc.vector.tensor_copy(dummy[:10, 8:16], bn1_b[:10, :])
                nc.sync.dma_start(out=out_d[:], in_=dummy[:10, :])
                nc.finalize()
                return nc

            # ------------- main pass: BN1+ReLU -> L2 -> stats ------------
            w2v = w2ps[:].rearrange("p (kp two h) -> p kp two h", kp=4, two=2)
            with (
                tc.tile_pool(name="acts", bufs=1) as acts,
                tc.tile_pool(name="sqp", bufs=1) as sqp,
                tc.tile_pool(name="ph2", bufs=2, space="PSUM") as ph2,
            ):
                for ci, (c0, cc) in enumerate(CH_LIST):
                    sl = slice(c0, c0 + cc)
                    a1t = [
                        acts.tile(
                            [128, CH], l2dt, tag=f"act{ci & 1}_{k}",
                            name=f"a1_{ci}_{k}",
                        )
                        for k in range(8)
                    ]
                    for m in range(8):
                        nc.scalar.activation(
                            a1t[m][:, :cc],
                            hbuf[m][:, sl],
                            AF.Relu,
                            bias=bn1_b[:, m : m + 1],
                            scale=bn1_s[:, m : m + 1],
                        )
                    for m2 in range(8):
                        hp = ph2.tile([128, CH], fp32, tag="ph2", name=f"h2_{ci}_{m2}")
                        msl = slice(m2 * 128, (m2 + 1) * 128)
                        if L2_FP8:
                            raise NotImplementedError("fp8 path disabled")
                        else:
                            for k in range(8):
                                for off, nn in _halves(cc):
                                    nc.tensor.matmul(
                                        hp[:, off : off + nn],
                                        lhsT=w2v[:, k // 2, k % 2, msl],
                                        rhs=a1t[k][:, off : off + nn],
                                        start=(k == 0),
                                        stop=(k == 7),
                                    )
                        # PSUM -> SBUF (bf16, in place over h1) + row-sum
                        nc.vector.tensor_scalar(
                            out=hbuf[m2][:, sl],
                            in0=hp[:, :cc],
                            scalar1=1.0,
                            scalar2=0.0,
                            op0=ALU.mult,
                            op1=ALU.add,
                            accum_out=suma[:, m2 * NCH + ci : m2 * NCH + ci + 1],
                        )
                        sq = sqp.tile(
                            [128, CH], bf16, tag=f"sq{m2 & 1}", name=f"sq_{ci}_{m2}"
                        )
                        if m2 < 3:
                            nc.scalar.activation(
                                sq[:, :cc],
                                hbuf[m2][:, sl],
                                AF.Square,
                                accum_out=sumq[:, m2 * NCH + ci : m2 * NCH + ci + 1],
                            )
                        else:
                            nc.vector.scalar_tensor_tensor(
                                out=sq[:, :cc],
                                in0=hbuf[m2][:, sl],
                                scalar=1.0,
                                in1=hbuf[m2][:, sl],
                                op0=ALU.mult,
                                op1=ALU.mult,
                                accum_out=sumq[:, m2 * NCH + ci : m2 * NCH + ci + 1],
                            )

                # ---------------- BN2 statistics ----------------
                st2sb = sqp.tile([128, 16], fp32, tag="st2sb", name="st2sb")
                sumav = suma[:].rearrange("p (m c) -> p m c", m=8)
                sumqv = sumq[:].rearrange("p (m c) -> p m c", m=8)
                nc.vector.reduce_sum(st2sb[:, 0:8], sumav, axis=X)
                nc.vector.reduce_sum(st2sb[:, 8:16], sumqv, axis=X)
                nc.sync.dma_start(out=cc2_in[:], in_=st2sb[:])
                nc.gpsimd.collective_compute(
                    "AllReduce", ALU.add, replica_groups=RG,
                    ins=[cc2_in[:].opt()], outs=[cc2_out[:].opt()],
                )
                nc.sync.dma_start(out=st2g[:], in_=cc2_out[:])

            # ---------------- BN2 parameters (vectorized) ----------------
            with tc.tile_pool(name="bnw2", bufs=1) as bnw2:
                mean8 = bnw2.tile([128, 8], fp32, tag="n8", name="mean8")
                var8b = bnw2.tile([128, 8], fp32, tag="v8", name="var8b")
                msq8b = bnw2.tile([128, 8], fp32, tag="m8", name="msq8b")
                sd8b = bnw2.tile([128, 8], fp32, tag="s8", name="sd8b")
                rstd8b = bnw2.tile([128, 8], fp32, tag="r8", name="rstd8b")
                t8b = bnw2.tile([128, 8], fp32, tag="t8", name="t8b")
                nc.vector.tensor_scalar_mul(mean8[:], st2g[:, 0:8], 1.0 / N_TOTAL)
                nc.vector.tensor_scalar_mul(var8b[:], st2g[:, 8:16], 1.0 / N_TOTAL)
                nc.vector.tensor_mul(msq8b[:], mean8[:], mean8[:])
                nc.vector.tensor_sub(var8b[:], var8b[:], msq8b[:])
                nc.scalar.activation(sd8b[:], var8b[:], AF.Sqrt, bias=eps_c[:])
                nc.vector.reciprocal(rstd8b[:], sd8b[:])
                nc.vector.tensor_mul(bn2_s[:], rstd8b[:], vcv[:, 4, :])
                nc.vector.tensor_mul(t8b[:], mean8[:], bn2_s[:])
                nc.vector.tensor_sub(bn2_b[:], vcv[:, 5, :], t8b[:])

            # ------ final pass A: BN2+ReLU -> L3 -> L4 -> logits ------
            wl1v = wl1ps[:].rearrange("p (kp two h) -> p kp two h", kp=4, two=2)
            with (
                tc.tile_pool(name="acts2", bufs=1) as acts2,
                tc.tile_pool(name="fsb", bufs=2) as fsb,
                tc.tile_pool(name="ph3", bufs=2, space="PSUM") as ph3,
                tc.tile_pool(name="plg", bufs=2, space="PSUM") as plg,
            ):
                for ci, (c0, cc) in enumerate(FCH_LIST):
                    sl = slice(c0, c0 + cc)
                    a2t = [
                        acts2.tile(
                            [128, FCH], l3dt, tag=f"act{ci & 1}_{k}",
                            name=f"a2_{ci}_{k}",
                        )
                        for k in range(8)
                    ]
                    for m in range(8):
                        if m < 6:
                            nc.scalar.activation(
                                a2t[m][:, :cc],
                                hbuf[m][:, sl],
                                AF.Relu,
                                bias=bn2_b[:, m : m + 1],
                                scale=bn2_s[:, m : m + 1],
                            )
                        else:
                            tmp = fsb.tile(
                                [128, FCH], bf16, tag=f"tmp{m}", name=f"tmp_{ci}_{m}"
                            )
                            nc.vector.tensor_scalar(
                                out=tmp[:, :cc],
                                in0=hbuf[m][:, sl],
                                scalar1=bn2_s[:, m : m + 1],
                                scalar2=bn2_b[:, m : m + 1],
                                op0=ALU.mult,
                                op1=ALU.add,
                            )
                            nc.vector.tensor_scalar_max(
                                a2t[m][:, :cc], tmp[:, :cc], 0.0
                            )
                    h3t = []
                    for m3 in range(2):
                        hp3 = ph3.tile(
                            [128, FCH], fp32, tag="ph3", name=f"h3_{ci}_{m3}"
                        )
                        msl = slice(m3 * 128, (m3 + 1) * 128)
                        for k in range(8):
                            nc.tensor.matmul(
                                hp3[:, :cc],
                                lhsT=wl1v[:, k // 2, k % 2, msl],
                                rhs=a2t[k][:, :cc],
                                start=(k == 0),
                                stop=(k == 7),
                            )
                        h3 = fsb.tile(
                            [128, FCH], bf16, tag=f"h3_{m3}", name=f"h3t_{ci}_{m3}"
                        )
                        nc.vector.tensor_scalar(
                            out=h3[:, :cc],
                            in0=hp3[:, :cc],
                            scalar1=vcv[:, 6, m3 : m3 + 1],
                            scalar2=0.0,
                            op0=ALU.add,
                            op1=ALU.max,
                        )
                        h3t.append(h3)
                    # L4: logits (pre-bl2) -> PSUM -> lgall (bf16)
                    lg = plg.tile([C, FCH], fp32, tag="lg", name=f"lg_{ci}")
                    for k in range(2):
                        nc.tensor.matmul(
                            lg[:, :cc],
                            lhsT=wl2ts[:, k * C : (k + 1) * C],
                            rhs=h3t[k][:, :cc],
                            start=(k == 0),
                            stop=(k == 1),
                        )
                    nc.vector.tensor_copy(lgall[:C, sl], lg[:, :cc])

            # ---------- batched log_softmax over [10, R] ----------
            with (
                tc.tile_pool(name="smx", bufs=2) as smx,
                tc.tile_pool(name="psmB", bufs=2, space="PSUM") as psmB,
            ):
                EW = 2048
                for o0 in range(0, R, EW):
                    ee = min(EW, R - o0)
                    nc.scalar.activation(
                        e_all[:C, o0 : o0 + ee],
                        lgall[:C, o0 : o0 + ee],
                        AF.Exp,
                        bias=bl2cs[:C, :],
                    )
                NF = len(FCH_LIST)
                lns_t = {}
                for step in range(NF + 1):
                    if step < NF:
                        c0, cc = FCH_LIST[step]
                        sm = psmB.tile(
                            [1, FCH], fp32, tag=f"sm{step & 1}", name=f"smB_{step}"
                        )
                        nc.tensor.matmul(
                            sm[:, :cc],
                            lhsT=ones_bf[:C, :],
                            rhs=e_all[:C, c0 : c0 + cc],
                            start=True,
                            stop=True,
                        )
                        lns = smx.tile(
                            [1, FCH], bf16, tag=f"lns{step & 1}", name=f"lnsB_{step}"
                        )
                        nc.scalar.activation(lns[:, :cc], sm[:, :cc], AF.Ln)
                        lns_t[step] = lns
                    if step >= 1:
                        ci = step - 1
                        c0, cc = FCH_LIST[ci]
                        sl = slice(c0, c0 + cc)
                        bc = psmB.tile(
                            [C, FCH], fp32, tag=f"bc{ci & 1}", name=f"bcB_{ci}"
                        )
                        nc.tensor.matmul(
                            bc[:, :cc],
                            lhsT=mones_bf[:, :C],
                            rhs=lns_t[ci][:, :cc],
                            start=True,
                            stop=True,
                        )
                        osb = smx.tile(
                            [C, FCH], fp32, tag=f"osb{ci & 1}", name=f"osbB_{ci}"
                        )
                        nc.vector.scalar_tensor_tensor(
                            out=osb[:, :cc],
                            in0=lgall[:C, sl],
                            scalar=bl2cs[:C, :],
                            in1=bc[:, :cc],
                            op0=ALU.add,
                            op1=ALU.add,
                        )
                        nc.sync.dma_start(out=out_d[:, sl], in_=osb[:, :cc])

    nc.finalize()
    return nc


def _get_nc():
    if "nc" not in _CACHE:
        _CACHE["nc"] = _build(os.environ.get("KERNEL_STAGE", "full"))
    return _CACHE["nc"]


def make_in_maps(inputs):
    """Host-side layout/dtype prep. Layout-only transforms (transpose, tile,
    pack, cast) - all math happens on device."""
    f32 = np.float32
    bf = ml_dtypes.bfloat16
    f8 = ml_dtypes.float8_e4m3

    x = np.ascontiguousarray(np.asarray(inputs["x"]), dtype=f32)
    W1 = np.asarray(inputs["W1"], dtype=f32)
    W2 = np.asarray(inputs["W2"], dtype=f32)
    Wl1 = np.asarray(inputs["Wl1"], dtype=f32)
    Wl2 = np.asarray(inputs["Wl2"], dtype=f32)

    w1f = np.ascontiguousarray(W1.T)  # [128, H] f32 (f32r on device)
    w1b = w1f.astype(bf)
    l2np = f8 if L2_FP8 else bf
    l3np = f8 if L3_FP8 else bf
    # [128, kp, two, m] packing (DoubleRow-compatible; plain reshape for bf16)
    w2p = np.ascontiguousarray(
        W2.T.reshape(4, 2, 128, H).transpose(2, 0, 1, 3).reshape(128, 8 * H)
    ).astype(l2np)
    wl1p = np.ascontiguousarray(
        Wl1.T.reshape(4, 2, 128, HM).transpose(2, 0, 1, 3).reshape(128, 8 * HM)
    ).astype(l3np)
    wl2t = np.ascontiguousarray(
        Wl2.T.reshape(2, 128, C).transpose(1, 0, 2).reshape(128, 2 * C)
    ).astype(bf)

    vecs = np.zeros((8, H), f32)
    vecs[1, :] = np.asarray(inputs["g1"], dtype=f32)
    vecs[2, :] = np.asarray(inputs["be1"], dtype=f32)
    vecs[4, :] = np.asarray(inputs["g2"], dtype=f32)
    vecs[5, :] = np.asarray(inputs["be2"], dtype=f32)
    vecs[6, :HM] = np.asarray(inputs["bl1"], dtype=f32)
    # vc[p, k, j] = vecs[j, k*128+p]
    vc = np.ascontiguousarray(
        vecs.T.reshape(8, 128, 8).transpose(1, 0, 2).reshape(128, 64)
    )
    bl2c = np.zeros((16, 1), f32)
    bl2c[:C, 0] = np.asarray(inputs["bl2"], dtype=f32)

    in_maps = []
    for i in range(NCORES):
        xs = x[i * R : (i + 1) * R]
        xt = np.ascontiguousarray(xs.T).astype(bf)
        x2 = np.zeros((NRT * 128, D1), f32)
        x2[:R, :DIN] = xs
        x2[:R, DIN] = 1.0
        x2 = np.ascontiguousarray(
            x2.reshape(NRT, 128, D1).transpose(1, 0, 2).reshape(128, NRT * D1)
        ).astype(bf)
        in_maps.append(
            {
                "x2": x2, "xT": xt, "w1f": w1f, "w1b": w1b, "w2p": w2p,
                "wl1p": wl1p, "wl2t": wl2t, "vc": vc, "bl2c": bl2c,
                "ones": np.ones((128, 1), f32),
                "mones": np.full((1, 16), -1.0, f32),
            }
        )
    return in_maps


def kernel(**inputs):
    from concourse.bass_utils import run_bass_kernel_spmd

    nc = _get_nc()
    in_maps = make_in_maps(inputs)
    res = run_bass_kernel_spmd(nc, in_maps, core_ids=list(range(NCORES)))
    return np.concatenate(
        [np.asarray(r["out"], dtype=np.float32).T for r in res.results], axis=0
    )


# revision 13
# speedup vs baseline: 1.5398x; 1.0607x over previous
"""Trainium2 Bass kernel for ChebyNet (K=1) forward pass.

ChebConv with K=1 reduces to a plain linear layer on the T0 (identity) term,
so edge_index / edge_weight never enter the math. The network is:

    h1 = x @ W1.T (+b1)           -> BN (train mode, over nodes) -> ReLU
    h2 = a1 @ W2.T (+b2)          -> BN -> ReLU
    h3 = relu(h2 @ Wl1.T + bl1)
    out = log_softmax(h3 @ Wl2.T + bl2, axis=1)

(b1/b2 cancel exactly inside train-mode BN and are dropped.)

Sharding: nodes (N=50000) split across 8 NeuronCores (6250 rows each).
Everything is computed feature-on-partition ([feat, rows]).

Design (vs the 516-593us v0 baseline):
 - All transposes / dtype packing on host: xT, x row-tiles (with a ones
   column for the column-sum), W1T, W2T/Wl1T/Wl2T, per-partition BN
   parameter columns. No on-device transposes.
 - bf16 operands everywhere on the matmul path (fp8 DoubleRow measured at
   rel_err 2.6-3.4e-2 for the K=1024 layers - over the 2e-2 gate).
 - BN1 stats analytically from the Gram matrix of x, projected locally to
   diag(W1 G W1^T) BEFORE the AllReduce -> payload [128,16] (8KB).
 - During the AR1 wait, L1 (pre-BN) is computed for all chunks into SBUF
   (bf16). The main pass applies BN1+ReLU, runs L2, and overwrites h1 with
   h2 *in place* in SBUF - h2 never spills to DRAM.
 - BN2 stats: sum(h2) via vector tensor_scalar accum_out during the
   PSUM->SBUF copy; sumsq(h2) split scalar(Square+accum)/vector(STT+accum).
 - log_softmax batched at the end ([10, R] feature-major) so the scalar
   activation table is not thrashed per chunk (Relu<->Exp<->Ln reloads cost
   1.3us each); -ln(sum) is broadcast via a K=1 matmul. Output is [10, R];
   host transposes to [R, 10] (layout-only, like the xT input).
"""

import os
import sys

sys.path.insert(0, "/opt/trn_rl_repo")

import numpy as np
import ml_dtypes

NCORES = 8
N_TOTAL = 50000
R = N_TOTAL // NCORES  # 6250 rows per core
DIN = 128
H = 1024
HM = 256
C = 10
BN_EPS = 1e-5

CH = 1024  # main-pass row chunk
FCH = 512  # final-pass row chunk
CH_LIST = [(i * CH, min(CH, R - i * CH)) for i in range((R + CH - 1) // CH)]
FCH_LIST = [(i * FCH, min(FCH, R - i * FCH)) for i in range((R + FCH - 1) // FCH)]
if os.environ.get("CH_LIMIT"):
    CH_LIST = CH_LIST[: int(os.environ["CH_LIMIT"])]
NCH = len(CH_LIST)

NRT = (R + 127) // 128  # 49 row tiles for the Gram matrix
D1 = DIN + 1  # x tile width incl the ones column

L2_FP8 = os.environ.get("L2_FP8", "0") == "1"
L3_FP8 = os.environ.get("L3_FP8", "0") == "1"

_CACHE = {}


def _halves(cc, step=512):
    out = []
    off = 0
    while off < cc:
        out.append((off, min(step, cc - off)))
        off += step
    return out


def _build(stage="full"):
    import concourse.bass as bass  # noqa: F401
    import concourse.tile as tile
    import concourse.mybir as mybir
    from concourse import bacc

    fp32 = mybir.dt.float32
    f32r = mybir.dt.float32r
    bf16 = mybir.dt.bfloat16
    fp8 = mybir.dt.float8e4
    AF = mybir.ActivationFunctionType
    ALU = mybir.AluOpType
    X = mybir.AxisListType.X
    DR = mybir.MatmulPerfMode.DoubleRow

    l2dt = fp8 if L2_FP8 else bf16
    l3dt = fp8 if L3_FP8 else bf16

    nc = bacc.Bacc(num_devices=NCORES, debug=False)

    x2_d = nc.dram_tensor("x2", [128, NRT * D1], bf16, kind="ExternalInput")
    xt_d = nc.dram_tensor("xT", [128, R], bf16, kind="ExternalInput")
    w1f_d = nc.dram_tensor("w1f", [128, H], f32r, kind="ExternalInput")
    w1b_d = nc.dram_tensor("w1b", [128, H], bf16, kind="ExternalInput")
    w2b_d = nc.dram_tensor("w2b", [128, 6 * H], bf16, kind="ExternalInput")
    w2f8_d = nc.dram_tensor("w2f8", [128, 2 * H], fp8, kind="ExternalInput")
    wl1p_d = nc.dram_tensor("wl1p", [128, 8 * HM], l3dt, kind="ExternalInput")
    wl2t_d = nc.dram_tensor("wl2t", [128, 2 * C], bf16, kind="ExternalInput")
    vc_d = nc.dram_tensor("vc", [128, 64], fp32, kind="ExternalInput")
    bl2c_d = nc.dram_tensor("bl2c", [16, 1], fp32, kind="ExternalInput")
    ones_d = nc.dram_tensor("ones", [128, 1], f32r, kind="ExternalInput")
    mones_d = nc.dram_tensor("mones", [1, 16], f32r, kind="ExternalInput")
    out_d = nc.dram_tensor("out", [C, R], fp32, kind="ExternalOutput")

    RG = [list(range(NCORES))]

    with tile.TileContext(nc) as tc:
        with (
            tc.tile_pool(name="persist", bufs=1) as persist,
            tc.tile_pool(name="dram", bufs=1, space="DRAM") as dram,
        ):
            # ---------------- persistent tiles -----------------
            hbuf = [
                persist.tile([128, R], bf16, tag=f"hb{m}", name=f"hbuf{m}")
                for m in range(8)
            ]
            w1bs = persist.tile([128, H], bf16, tag="w1bs", name="w1bs")
            w2bs = persist.tile([128, 6 * H], bf16, tag="w2bs", name="w2bs")
            w2f8s = persist.tile([128, 2 * H], fp8, tag="w2f8s", name="w2f8s")
            wl1ps = persist.tile([128, 8 * HM], l3dt, tag="wl1ps", name="wl1ps")
            wl2ts = persist.tile([128, 2 * C], bf16, tag="wl2ts", name="wl2ts")
            vcs = persist.tile([128, 64], fp32, tag="vcs", name="vcs")
            bl2cs = persist.tile([16, 1], fp32, tag="bl2cs", name="bl2cs")
            ones_r = persist.tile([128, 1], f32r, tag="ones_r", name="ones_r")
            mones10 = persist.tile([1, 16], f32r, tag="mones", name="mones10")
            bn1_s = persist.tile([128, 8], fp32, tag="bn1s", name="bn1_s")
            bn1_b = persist.tile([128, 8], fp32, tag="bn1b", name="bn1_b")
            bn2_s = persist.tile([128, 8], fp32, tag="bn2s", name="bn2_s")
            bn2_b = persist.tile([128, 8], fp32, tag="bn2b", name="bn2_b")
            eps_c = persist.tile([128, 1], fp32, tag="epsc", name="eps_c")
            suma = persist.tile([128, 8 * NCH], fp32, tag="suma", name="suma")
            sumq = persist.tile([128, 8 * NCH], fp32, tag="sumq", name="sumq")
            st1g = persist.tile([128, 16], fp32, tag="st1g", name="st1g")
            st2g = persist.tile([128, 16], fp32, tag="st2g", name="st2g")
            ones_bf = persist.tile([128, 1], bf16, tag="onesbf", name="ones_bf")
            mones_bf = persist.tile([1, 16], bf16, tag="monesbf", name="mones_bf")
            lgall = persist.tile([16, R], bf16, tag="lgall", name="lgall")
            e_all = persist.tile([16, R], bf16, tag="e_all", name="e_all")

            cc1_in = dram.tile([128, 16], fp32, name="cc1_in")
            cc1_out = dram.tile([128, 16], fp32, name="cc1_out")
            cc2_in = dram.tile([128, 16], fp32, name="cc2_in")
            cc2_out = dram.tile([128, 16], fp32, name="cc2_out")

            nc.vector.memset(eps_c[:], BN_EPS)
            nc.vector.memset(ones_bf[:], 1.0)
            nc.vector.memset(mones_bf[:], -1.0)
            nc.scalar.dma_start(out=ones_r[:], in_=ones_d[:])
            nc.scalar.dma_start(out=mones10[:], in_=mones_d[:])

            # per-partition views of the BN parameter columns
            vcv = vcs[:].rearrange("p (m j) -> p j m", j=8)  # [128, j, m]

            # ============ startup + prefill (xT lives only here) ============
            with tc.tile_pool(name="xtp", bufs=1) as xtp:
                xTs = xtp.tile([128, R], bf16, tag="xTs", name="xTs")

                with (
                    tc.tile_pool(name="startsb", bufs=1) as startsb,
                    tc.tile_pool(name="startps", bufs=1, space="PSUM") as startps,
                ):
                    x2s = startsb.tile(
                        [128, NRT * D1], bf16, tag="x2s", name="x2s"
                    )
                    w1fr = startsb.tile([128, H], f32r, tag="w1fr", name="w1fr")
                    v_r = startsb.tile([128, H], bf16, tag="v_r", name="v_r")
                    mean_r = startsb.tile([128, 1], bf16, tag="mean_r", name="mean_r")
                    st1sb = startsb.tile([128, 16], fp32, tag="st1sb", name="st1sb")

                    # big loads on the sync DMA queue; small ones on scalar
                    nc.sync.dma_start(out=x2s[:], in_=x2_d[:])
                    nc.sync.dma_start(out=xTs[:], in_=xt_d[:])
                    nc.sync.dma_start(out=w2bs[:], in_=w2b_d[:])
                    nc.sync.dma_start(out=w2f8s[:], in_=w2f8_d[:])
                    nc.sync.dma_start(out=wl1ps[:], in_=wl1p_d[:])
                    nc.scalar.dma_start(out=w1fr[:], in_=w1f_d[:])
                    nc.scalar.dma_start(out=w1bs[:], in_=w1b_d[:])
                    nc.scalar.dma_start(out=vcs[:], in_=vc_d[:])
                    nc.scalar.dma_start(out=wl2ts[:], in_=wl2t_d[:])
                    nc.scalar.dma_start(out=bl2cs[:], in_=bl2c_d[:])

                    # Gram matrix of x (incl ones column -> column sums)
                    gram_ps = startps.tile(
                        [128, D1], fp32, tag="gram", name="gram_ps"
                    )
                    for t in range(NRT):
                        o = t * D1
                        nc.tensor.matmul(
                            gram_ps[:],
                            lhsT=x2s[:, o : o + DIN],
                            rhs=x2s[:, o : o + D1],
                            start=(t == 0),
                            stop=(t == NRT - 1),
                        )
                    gram_r = startsb.tile(
                        [128, D1], f32r, tag="gram_r", name="gram_r"
                    )
                    nc.vector.tensor_copy(gram_r[:], gram_ps[:])
                    nc.scalar.mul(mean_r[:], gram_r[:, DIN : D1], 1.0 / N_TOTAL)

                    # P = G @ W1T ; V = W1T*P ; e2[f]=colsum(V) ; wxm = W1T.T mean
                    st1_ps = startps.tile(
                        [128, 16], fp32, tag="st1ps", name="st1_ps"
                    )
                    for hf in range(2):
                        sl = slice(hf * 512, (hf + 1) * 512)
                        p_ps = startps.tile(
                            [128, 512], fp32, tag=f"pps{hf}", name=f"p_ps{hf}"
                        )
                        nc.tensor.matmul(
                            p_ps[:], lhsT=gram_r[:, 0:DIN], rhs=w1fr[:, sl],
                            start=True, stop=True,
                        )
                        nc.vector.tensor_mul(v_r[:, sl], w1fr[:, sl], p_ps[:])
                    for m in range(8):
                        sl = slice(m * 128, (m + 1) * 128)
                        nc.tensor.matmul(
                            st1_ps[:, m : m + 1], lhsT=v_r[:, sl], rhs=ones_bf[:],
                            start=True, stop=True,
                        )
                        nc.tensor.matmul(
                            st1_ps[:, 8 + m : 9 + m], lhsT=w1bs[:, sl],
                            rhs=mean_r[:], start=True, stop=True,
                        )
                    nc.vector.tensor_copy(st1sb[:], st1_ps[:])
                    nc.sync.dma_start(out=cc1_in[:], in_=st1sb[:])
                    nc.gpsimd.collective_compute(
                        "AllReduce", ALU.add, replica_groups=RG,
                        ins=[cc1_in[:].opt()], outs=[cc1_out[:].opt()],
                    )
                    nc.sync.dma_start(out=st1g[:], in_=cc1_out[:])

                # -------- L1 prefill (runs during the AllReduce wait) -------
                with tc.tile_pool(name="ph1", bufs=4, space="PSUM") as ph1:
                    for ci, (c0, cc) in enumerate(CH_LIST):
                        for m in range(8):
                            hp = ph1.tile(
                                [128, CH], fp32, tag="ph1", name=f"h1_{ci}_{m}"
                            )
                            for off, nn in _halves(cc):
                                nc.tensor.matmul(
                                    hp[:, off : off + nn],
                                    lhsT=w1bs[:, m * 128 : (m + 1) * 128],
                                    rhs=xTs[:, c0 + off : c0 + off + nn],
                                    start=True,
                                    stop=True,
                                )
                            if m % 2 == 0:
                                nc.scalar.copy(
                                    hbuf[m][:, c0 : c0 + cc], hp[:, :cc]
                                )
                            else:
                                nc.vector.tensor_copy(
                                    hbuf[m][:, c0 : c0 + cc], hp[:, :cc]
                                )

            # ---------------- BN1 parameters (vectorized) ----------------
            with tc.tile_pool(name="bnw", bufs=1) as bnw:
                var8 = bnw.tile([128, 8], fp32, tag="v8", name="var8")
                msq8 = bnw.tile([128, 8], fp32, tag="m8", name="msq8")
                sd8 = bnw.tile([128, 8], fp32, tag="s8", name="sd8")
                rstd8 = bnw.tile([128, 8], fp32, tag="r8", name="rstd8")
                t8 = bnw.tile([128, 8], fp32, tag="t8", name="t8")
                nc.vector.tensor_scalar_mul(var8[:], st1g[:, 0:8], 1.0 / N_TOTAL)
                nc.vector.tensor_mul(msq8[:], st1g[:, 8:16], st1g[:, 8:16])
                nc.vector.tensor_sub(var8[:], var8[:], msq8[:])
                nc.scalar.activation(sd8[:], var8[:], AF.Sqrt, bias=eps_c[:])
                nc.vector.reciprocal(rstd8[:], sd8[:])
                nc.vector.tensor_mul(bn1_s[:], rstd8[:], vcv[:, 1, :])
                nc.vector.tensor_mul(t8[:], st1g[:, 8:16], bn1_s[:])
                nc.vector.tensor_sub(bn1_b[:], vcv[:, 2, :], t8[:])

            if stage == "s1":
                dummy = persist.tile([16, R], fp32, tag="dummy", name="dummy")
                nc.vector.memset(dummy[:], 0.0)
                nc.vector.tensor_copy(dummy[:10, 0:8], bn1_s[:10, :])
                nc.vector.tensor_copy(dummy[:10, 8:16], bn1_b[:10, :])
                nc.sync.dma_start(out=out_d[:], in_=dummy[:10, :])
                nc.finalize()
                return nc

            # ------------- main pass: BN1+ReLU -> L2 -> stats ------------
            w2bv = w2bs[:].rearrange("p (k h) -> p k h", k=6)
            w2f8v = w2f8s[:].rearrange("p (two h) -> p two h", two=2)
            with (
                tc.tile_pool(name="acts", bufs=1) as acts,
                tc.tile_pool(name="sqp", bufs=1) as sqp,
                tc.tile_pool(name="ph2", bufs=2, space="PSUM") as ph2,
            ):
                for ci, (c0, cc) in enumerate(CH_LIST):
                    sl = slice(c0, c0 + cc)
                    a1t = [
                        acts.tile(
                            [128, CH], bf16, tag=f"act{ci & 1}_{k}",
                            name=f"a1_{ci}_{k}",
                        )
                        for k in range(6)
                    ]
                    a1f8 = acts.tile(
                        [128, 2 * CH], fp8, tag=f"actf8{ci & 1}", name=f"a1f8_{ci}"
                    )
                    a1f8v = a1f8[:].rearrange("p (two c) -> p two c", two=2)
                    for m in range(8):
                        dst = (
                            a1t[m][:, :cc]
                            if m < 6
                            else a1f8[:, (m - 6) * CH : (m - 6) * CH + cc]
                        )
                        nc.scalar.activation(
                            dst,
                            hbuf[m][:, sl],
                            AF.Relu,
                            bias=bn1_b[:, m : m + 1],
                            scale=bn1_s[:, m : m + 1],
                        )
                    for m2 in range(8):
                        hp = ph2.tile([128, CH], fp32, tag="ph2", name=f"h2_{ci}_{m2}")
                        msl = slice(m2 * 128, (m2 + 1) * 128)
                        for k in range(6):
                            for off, nn in _halves(cc):
                                nc.tensor.matmul(
                                    hp[:, off : off + nn],
                                    lhsT=w2bv[:, k, msl],
                                    rhs=a1t[k][:, off : off + nn],
                                    start=(k == 0),
                                    stop=False,
                                )
                        for off, nn in _halves(cc):
                            nc.tensor.matmul(
                                hp[:, off : off + nn],
                                lhsT=w2f8v[:, :, msl],
                                rhs=a1f8v[:, :, off : off + nn],
                                start=False,
                                stop=True,
                                perf_mode=DR,
                            )
                        # PSUM -> SBUF (bf16, in place over h1) + row-sum
                        nc.vector.tensor_scalar(
                            out=hbuf[m2][:, sl],
                            in0=hp[:, :cc],
                            scalar1=1.0,
                            scalar2=0.0,
                            op0=ALU.mult,
                            op1=ALU.add,
                            accum_out=suma[:, m2 * NCH + ci : m2 * NCH + ci + 1],
                        )
                        sq = sqp.tile(
                            [128, CH], bf16, tag=f"sq{m2 & 1}", name=f"sq_{ci}_{m2}"
                        )
                        if m2 < 3:
                            nc.scalar.activation(
                                sq[:, :cc],
                                hbuf[m2][:, sl],
                                AF.Square,
                                accum_out=sumq[:, m2 * NCH + ci : m2 * NCH + ci + 1],
                            )
                        else:
                            nc.vector.scalar_tensor_tensor(
                                out=sq[:, :cc],
                                in0=hbuf[m2][:, sl],
                                scalar=1.0,
                                in1=hbuf[m2][:, sl],
                                op0=ALU.mult,
                                op1=ALU.mult,
                                accum_out=sumq[:, m2 * NCH + ci : m2 * NCH + ci + 1],
                            )

                # ---------------- BN2 statistics ----------------
                st2sb = sqp.tile([128, 16], fp32, tag="st2sb", name="st2sb")
                sumav = suma[:].rearrange("p (m c) -> p m c", m=8)
                sumqv = sumq[:].rearrange("p (m c) -> p m c", m=8)
                nc.vector.reduce_sum(st2sb[:, 0:8], sumav, axis=X)
                nc.vector.reduce_sum(st2sb[:, 8:16], sumqv, axis=X)
                nc.sync.dma_start(out=cc2_in[:], in_=st2sb[:])
                nc.gpsimd.collective_compute(
                    "AllReduce", ALU.add, replica_groups=RG,
                    ins=[cc2_in[:].opt()], outs=[cc2_out[:].opt()],
                )
                nc.sync.dma_start(out=st2g[:], in_=cc2_out[:])

            # ---------------- BN2 parameters (vectorized) ----------------
            with tc.tile_pool(name="bnw2", bufs=1) as bnw2:
                mean8 = bnw2.tile([128, 8], fp32, tag="n8", name="mean8")
                var8b = bnw2.tile([128, 8], fp32, tag="v8", name="var8b")
                msq8b = bnw2.tile([128, 8], fp32, tag="m8", name="msq8b")
                sd8b = bnw2.tile([128, 8], fp32, tag="s8", name="sd8b")
                rstd8b = bnw2.tile([128, 8], fp32, tag="r8", name="rstd8b")
                t8b = bnw2.tile([128, 8], fp32, tag="t8", name="t8b")
                nc.vector.tensor_scalar_mul(mean8[:], st2g[:, 0:8], 1.0 / N_TOTAL)
                nc.vector.tensor_scalar_mul(var8b[:], st2g[:, 8:16], 1.0 / N_TOTAL)
                nc.vector.tensor_mul(msq8b[:], mean8[:], mean8[:])
                nc.vector.tensor_sub(var8b[:], var8b[:], msq8b[:])
                nc.scalar.activation(sd8b[:], var8b[:], AF.Sqrt, bias=eps_c[:])
                nc.vector.reciprocal(rstd8b[:], sd8b[:])
                nc.vector.tensor_mul(bn2_s[:], rstd8b[:], vcv[:, 4, :])
                nc.vector.tensor_mul(t8b[:], mean8[:], bn2_s[:])
                nc.vector.tensor_sub(bn2_b[:], vcv[:, 5, :], t8b[:])

            # ------ final pass A: BN2+ReLU -> L3 -> L4 -> logits ------
            wl1v = wl1ps[:].rearrange("p (kp two h) -> p kp two h", kp=4, two=2)
            with (
                tc.tile_pool(name="acts2", bufs=1) as acts2,
                tc.tile_pool(name="fsb", bufs=2) as fsb,
                tc.tile_pool(name="ph3", bufs=2, space="PSUM") as ph3,
                tc.tile_pool(name="plg", bufs=2, space="PSUM") as plg,
            ):
                for ci, (c0, cc) in enumerate(FCH_LIST):
                    sl = slice(c0, c0 + cc)
                    a2t = [
                        acts2.tile(
                            [128, FCH], l3dt, tag=f"act{ci & 1}_{k}",
                            name=f"a2_{ci}_{k}",
                        )
                        for k in range(8)
                    ]
                    for m in range(8):
                        if m < 6:
                            nc.scalar.activation(
                                a2t[m][:, :cc],
                                hbuf[m][:, sl],
                                AF.Relu,
                                bias=bn2_b[:, m : m + 1],
                                scale=bn2_s[:, m : m + 1],
                            )
                        else:
                            tmp = fsb.tile(
                                [128, FCH], bf16, tag=f"tmp{m}", name=f"tmp_{ci}_{m}"
                            )
                            nc.vector.tensor_scalar(
                                out=tmp[:, :cc],
                                in0=hbuf[m][:, sl],
                                scalar1=bn2_s[:, m : m + 1],
                                scalar2=bn2_b[:, m : m + 1],
                                op0=ALU.mult,
                                op1=ALU.add,
                            )
                            nc.vector.tensor_scalar_max(
                                a2t[m][:, :cc], tmp[:, :cc], 0.0
                            )
                    h3t = []
                    for m3 in range(2):
                        hp3 = ph3.tile(
                            [128, FCH], fp32, tag="ph3", name=f"h3_{ci}_{m3}"
                        )
                        msl = slice(m3 * 128, (m3 + 1) * 128)
                        for k in range(8):
                            nc.tensor.matmul(
                                hp3[:, :cc],
                                lhsT=wl1v[:, k // 2, k % 2, msl],
                                rhs=a2t[k][:, :cc],
                                start=(k == 0),
                                stop=(k == 7),
                            )
                        h3 = fsb.tile(
                            [128, FCH], bf16, tag=f"h3_{m3}", name=f"h3t_{ci}_{m3}"
                        )
                        nc.vector.tensor_scalar(
                            out=h3[:, :cc],
                            in0=hp3[:, :cc],
                            scalar1=vcv[:, 6, m3 : m3 + 1],
                            scalar2=0.0,
                            op0=ALU.add,
                            op1=ALU.max,
                        )
                        h3t.append(h3)
                    # L4: logits (pre-bl2) -> PSUM -> lgall (bf16)
                    lg = plg.tile([C, FCH], fp32, tag="lg", name=f"lg_{ci}")
                    for k in range(2):
                        nc.tensor.matmul(
                            lg[:, :cc],
                            lhsT=wl2ts[:, k * C : (k + 1) * C],
                            rhs=h3t[k][:, :cc],
                            start=(k == 0),
                            stop=(k == 1),
                        )
                    nc.vector.tensor_copy(lgall[:C, sl], lg[:, :cc])

            # ---------- batched log_softmax over [10, R] ----------
            with (
                tc.tile_pool(name="smx", bufs=2) as smx,
                tc.tile_pool(name="psmB", bufs=2, space="PSUM") as psmB,
            ):
                EW = 2048
                for o0 in range(0, R, EW):
                    ee = min(EW, R - o0)
                    nc.scalar.activation(
                        e_all[:C, o0 : o0 + ee],
                        lgall[:C, o0 : o0 + ee],
                        AF.Exp,
                        bias=bl2cs[:C, :],
                    )
                NF = len(FCH_LIST)
                lns_t = {}
                for step in range(NF + 1):
                    if step < NF:
                        c0, cc = FCH_LIST[step]
                        sm = psmB.tile(
                            [1, FCH], fp32, tag=f"sm{step & 1}", name=f"smB_{step}"
                        )
                        nc.tensor.matmul(
                            sm[:, :cc],
                            lhsT=ones_bf[:C, :],
                            rhs=e_all[:C, c0 : c0 + cc],
                            start=True,
                            stop=True,
                        )
                        lns = smx.tile(
                            [1, FCH], bf16, tag=f"lns{step & 1}", name=f"lnsB_{step}"
                        )
                        nc.scalar.activation(lns[:, :cc], sm[:, :cc], AF.Ln)
                        lns_t[step] = lns
                    if step >= 1:
                        ci = step - 1
                        c0, cc = FCH_LIST[ci]
                        sl = slice(c0, c0 + cc)
                        bc = psmB.tile(
                            [C, FCH], fp32, tag=f"bc{ci & 1}", name=f"bcB_{ci}"
                        )
                        nc.tensor.matmul(
                            bc[:, :cc],
                            lhsT=mones_bf[:, :C],
                            rhs=lns_t[ci][:, :cc],
                            start=True,
                            stop=True,
                        )
                        osb = smx.tile(
                            [C, FCH], fp32, tag=f"osb{ci & 1}", name=f"osbB_{ci}"
                        )
                        nc.vector.scalar_tensor_tensor(
                            out=osb[:, :cc],
                            in0=lgall[:C, sl],
                            scalar=bl2cs[:C, :],
                            in1=bc[:, :cc],
                            op0=ALU.add,
                            op1=ALU.add,
                        )
                        nc.sync.dma_start(out=out_d[:, sl], in_=osb[:, :cc])

    nc.finalize()
    return nc


def _get_nc():
    if "nc" not in _CACHE:
        _CACHE["nc"] = _build(os.environ.get("KERNEL_STAGE", "full"))
    return _CACHE["nc"]


def make_in_maps(inputs):
    """Host-side layout/dtype prep. Layout-only transforms (transpose, tile,
    pack, cast) - all math happens on device."""
    f32 = np.float32
    bf = ml_dtypes.bfloat16
    f8 = ml_dtypes.float8_e4m3

    x = np.ascontiguousarray(np.asarray(inputs["x"]), dtype=f32)
    W1 = np.asarray(inputs["W1"], dtype=f32)
    W2 = np.asarray(inputs["W2"], dtype=f32)
    Wl1 = np.asarray(inputs["Wl1"], dtype=f32)
    Wl2 = np.asarray(inputs["Wl2"], dtype=f32)

    w1f = np.ascontiguousarray(W1.T)  # [128, H] f32 (f32r on device)
    w1b = w1f.astype(bf)
    l3np = f8 if L3_FP8 else bf
    # L2: first 768 contraction dims bf16, last 256 packed fp8 (DoubleRow)
    w2b = np.ascontiguousarray(
        W2.T[:768].reshape(6, 128, H).transpose(1, 0, 2).reshape(128, 6 * H)
    ).astype(bf)
    w2f8 = np.ascontiguousarray(
        W2.T[768:].reshape(2, 128, H).transpose(1, 0, 2).reshape(128, 2 * H)
    ).astype(f8)
    wl1p = np.ascontiguousarray(
        Wl1.T.reshape(4, 2, 128, HM).transpose(2, 0, 1, 3).reshape(128, 8 * HM)
    ).astype(l3np)
    wl2t = np.ascontiguousarray(
        Wl2.T.reshape(2, 128, C).transpose(1, 0, 2).reshape(128, 2 * C)
    ).astype(bf)

    vecs = np.zeros((8, H), f32)
    vecs[1, :] = np.asarray(inputs["g1"], dtype=f32)
    vecs[2, :] = np.asarray(inputs["be1"], dtype=f32)
    vecs[4, :] = np.asarray(inputs["g2"], dtype=f32)
    vecs[5, :] = np.asarray(inputs["be2"], dtype=f32)
    vecs[6, :HM] = np.asarray(inputs["bl1"], dtype=f32)
    # vc[p, k, j] = vecs[j, k*128+p]
    vc = np.ascontiguousarray(
        vecs.T.reshape(8, 128, 8).transpose(1, 0, 2).reshape(128, 64)
    )
    bl2c = np.zeros((16, 1), f32)
    bl2c[:C, 0] = np.asarray(inputs["bl2"], dtype=f32)

    in_maps = []
    for i in range(NCORES):
        xs = x[i * R : (i + 1) * R]
        xt = np.ascontiguousarray(xs.T).astype(bf)
        x2 = np.zeros((NRT * 128, D1), f32)
        x2[:R, :DIN] = xs
        x2[:R, DIN] = 1.0
        x2 = np.ascontiguousarray(
            x2.reshape(NRT, 128, D1).transpose(1, 0, 2).reshape(128, NRT * D1)
        ).astype(bf)
        in_maps.append(
            {
                "x2": x2, "xT": xt, "w1f": w1f, "w1b": w1b,
                "w2b": w2b, "w2f8": w2f8,
                "wl1p": wl1p, "wl2t": wl2t, "vc": vc, "bl2c": bl2c,
                "ones": np.ones((128, 1), f32),
                "mones": np.full((1, 16), -1.0, f32),
            }
        )
    return in_maps


def kernel(**inputs):
    from concourse.bass_utils import run_bass_kernel_spmd

    nc = _get_nc()
    in_maps = make_in_maps(inputs)
    res = run_bass_kernel_spmd(nc, in_maps, core_ids=list(range(NCORES)))
    return np.concatenate(
        [np.asarray(r["out"], dtype=np.float32).T for r in res.results], axis=0
    )
